# revision 45
# baseline (speedup 1.0000x reference)
"""MLA-style attention kernel for 8 TRN2 NeuronCores, v3.

Sharding: core c -> batch b = c//4, heads r*4..r*4+3 where r = c%4.
The latent down-projections are REPLICATED within each 4-core batch
group (no collective, no cross-core dependency): each core computes the
full-T latents c_q/c_kv/k_r from the full x[b], then its 4 heads'
attention and a partial output projection summed on the host.

All activations stay SBUF-resident in a transposed [feature, T] layout.
Down/up-projections run in bf16 (PSUM fp32 accumulate).  Scores use
fp8e4 with DoubleRow perf mode: q/k packed as [128, 2, T] fp8 where
slot 0 holds the 128 content dims and slot 1 rows 0:64 hold the roped
rope dims (planar re/im), rows 64:128 zero.  One DoubleRow matmul per
512x128 score subtile (4x fewer PE cycles than two f32r matmuls).
Softmax denominators use N=1 ones-column matmuls accumulating into a
[128q, 4] PSUM tile (start=False onto memset zeros -- a start=True
would wipe sibling columns through the 2KB zero-region), then
transpose + reciprocal + selector broadcast matmul for normalization.
Diagonal causal blocks shrink the score matmul to the valid q-range,
memset the dead Pt columns, and add a fixed 128-wide triangular mask.
exp() pipelines two subtiles deep; the normalization tail of head h and
the output projection of chunk tq-1 are emitted under the NEXT head's
score loop so PE never drains while Act works.  V is produced directly
in natural [t, hs] layout (lhsT = ckv^T): no transposes after phase A.
Both hardware DMA queues are used: SP for W_dq/W_dkv/x/output, Act for
cos/sin, W_u/W_qr/W_o, mask, and the SBUF-to-SBUF fp8 slot copies.
"""
import math
from collections import deque
import numpy as np

import concourse.bass as bass
import concourse.bacc as bacc
import concourse.mybir as mybir
import concourse.tile as tile
from concourse.bass_utils import run_bass_kernel_spmd

F32 = mybir.dt.float32
F32R = mybir.dt.float32r
BF16 = mybir.dt.bfloat16
F8 = mybir.dt.float8e4
Exp = mybir.ActivationFunctionType.Exp
DR = mybir.MatmulPerfMode.DoubleRow

B, T, C = 2, 2048, 2048
H = 16
HS = 128
NL = 512
RHD = 64
HLOC = 4              # heads per core
P = 128
NNL = NL // P         # 4
NCT = C // P          # 16
TCH = 512
NCH = T // TCH        # 4
SCALE = 1.0 / math.sqrt(HS + RHD)
NEG = -1.0e30

_NC_CACHE = {}


def _deint(ap2d):
    # [p, 2d] -> (evens [p, d], odds [p, d]) along the free dim
    rr = ap2d.rearrange("p (d two) -> p two d", two=2)
    return rr[:, 0, :], rr[:, 1, :]


def build():
    nc = bacc.Bacc("TRN2", target_bir_lowering=False, debug=False, num_devices=8)

    x_ext = nc.dram_tensor("x", [T, C], F32R, kind="ExternalInput")
    wdq_ext = nc.dram_tensor("wdq", [NL, C], F32R, kind="ExternalInput")
    wdkv_ext = nc.dram_tensor("wdkv", [NL, C], F32R, kind="ExternalInput")
    wkr_ext = nc.dram_tensor("wkr", [RHD, C], F32R, kind="ExternalInput")
    wuq_ext = nc.dram_tensor("wuq", [HLOC * HS, NL], F32R, kind="ExternalInput")
    wuk_ext = nc.dram_tensor("wuk", [HLOC * HS, NL], F32R, kind="ExternalInput")
    wuv_ext = nc.dram_tensor("wuv", [HLOC * HS, NL], F32R, kind="ExternalInput")
    wqr_ext = nc.dram_tensor("wqr", [HLOC * RHD, NL], F32R, kind="ExternalInput")
    wo_ext = nc.dram_tensor("wo", [C, HLOC * HS], F32R, kind="ExternalInput")
    cos_ext = nc.dram_tensor("cos", [T, RHD // 2], F32R, kind="ExternalInput")
    sin_ext = nc.dram_tensor("sin", [T, RHD // 2], F32R, kind="ExternalInput")
    out_ext = nc.dram_tensor("out", [C, T], F32, kind="ExternalOutput")

    ident_dram = nc.inline_tensor(np.eye(P, dtype=np.float32), name="identc")
    # triangular mask for the 128-wide diagonal band of shrunk S^T tiles
    m = np.full((P, 896), NEG, dtype=np.float32)
    for jj in range(P):
        m[jj, 384 + jj:] = 0.0
    masks_dram = nc.inline_tensor(m, name="maskc")
    # row selector for the 1/den broadcast: sel4[k, qq*128+j] = (k == qq)
    sel = np.zeros((4, 512), dtype=np.float32)
    for qq in range(4):
        sel[qq, qq * P:(qq + 1) * P] = 1.0
    sel4_dram = nc.inline_tensor(sel, name="sel4c")

    with tile.TileContext(nc) as tc:
        with tc.tile_pool(name="pers", bufs=1) as pers:
            ptp_cell = [None]
            ident = pers.tile([P, P], F32R, tag="ident", name="ident")
            nc.sync.dma_start(ident[:], ident_dram.ap().bitcast(F32R))
            maskbuf = pers.tile([P, 896], BF16, tag="maskbuf", name="maskbuf")
            nc.gpsimd.dma_start(out=maskbuf[:], in_=masks_dram.ap())
            onescol = pers.tile([P, 1], BF16, tag="onescol", name="onescol")
            nc.vector.memset(onescol[:], 1.0)
            identb = pers.tile([P, P], BF16, tag="identb", name="identb")
            nc.vector.tensor_copy(identb[:], ident[:])
            sel4 = pers.tile([4, TCH], BF16, tag="sel4", name="sel4")
            nc.gpsimd.dma_start(out=sel4[:], in_=sel4_dram.ap())

            # rope tables, cos/sin duplicated on all four 32-row groups
            ca4 = pers.tile([P, T], BF16, tag="ca4", name="ca4")
            sa4 = pers.tile([P, T], BF16, tag="sa4", name="sa4")

            # full-T latents (bf16, [feat, T])
            cqT = [pers.tile([P, T], BF16, tag=f"cqT{i}", name=f"cqT{i}")
                   for i in range(NNL)]
            ckvT = [pers.tile([P, T], BF16, tag=f"ckvT{i}", name=f"ckvT{i}")
                    for i in range(NNL)]
            krraw = pers.tile([RHD, T], BF16, tag="krraw", name="krraw")

            _ecnt = [0]

            def ecopy(dst, src, pin=None):
                """PSUM->SBUF evacuation copy, alternating Act/DVE."""
                _ecnt[0] += 1
                eng = pin if pin else ("act" if _ecnt[0] % 2 else "dve")
                if eng == "act":
                    nc.scalar.copy(dst, src)
                else:
                    nc.vector.tensor_copy(dst, src)

            def transpose_pair_into(dst_ap, srcA, srcB, pin=None):
                bf = srcA.dtype == BF16
                tp2 = ptp_cell[0].tile([P, 2 * P], BF16 if bf else F32R,
                                       tag="tpb" if bf else "tp", name="tp")
                idn = identb if bf else ident
                nc.tensor.transpose(tp2[:, 0:P], srcA, idn[:])
                nc.tensor.transpose(tp2[:, P:2 * P], srcB, idn[:])
                ecopy(dst_ap, tp2[:], pin=pin)

            # ============ phase A: x chunks + all weight prep, interleaved ==
            with tc.tile_pool(name="pb", bufs=1) as pb:
                # -- persistent-ish weight destinations (pb outlives phase A)
                wuqT = [pb.tile([P, HLOC * HS], BF16, tag=f"wuqT{i}",
                                name=f"wuqT{i}") for i in range(NNL)]
                wukT = [pb.tile([P, HLOC * HS], BF16, tag=f"wukT{i}",
                                name=f"wukT{i}") for i in range(NNL)]
                wuvT = [pb.tile([P, HLOC * HS], BF16, tag=f"wuvT{i}",
                                name=f"wuvT{i}") for i in range(NNL)]
                wqrT = [[pb.tile([P, P], BF16, tag=f"wqrT{g}{i}",
                                 name=f"wqrT{g}{i}") for i in range(NNL)]
                        for g in range(2)]
                woT = [pb.tile([P, C], BF16, tag=f"woT{i}", name=f"woT{i}")
                       for i in range(HLOC)]

                pa_ctx = (
                    tc.tile_pool(name="pa", bufs=1),
                    tc.tile_pool(name="pacc", bufs=1, space="PSUM"),
                    tc.tile_pool(name="ptpA", bufs=3, space="PSUM"),
                )
                pa = pa_ctx[0].__enter__()
                pacc = pa_ctx[1].__enter__()
                ptp_cell[0] = pa_ctx[2].__enter__()

                wdqT = [pa.tile([P, NL], BF16, tag=f"wdqT{i}", name=f"wdqT{i}")
                        for i in range(NCT)]
                wdkvT = [pa.tile([P, NL], BF16, tag=f"wdkvT{i}",
                                 name=f"wdkvT{i}") for i in range(NCT)]
                wkrT = [pa.tile([P, RHD], BF16, tag=f"wkrT{i}", name=f"wkrT{i}")
                        for i in range(NCT)]
                xT = [pa.tile([P, TCH], BF16, tag=f"xT{i}", name=f"xT{i}")
                      for i in range(NCT)]

                def x_chunk_transpose(tch):
                    t0 = tch * TCH
                    for sp in range(2):
                        rA = slice(t0 + 2 * sp * P, t0 + (2 * sp + 1) * P)
                        rB = slice(t0 + (2 * sp + 1) * P, t0 + (2 * sp + 2) * P)
                        for hf in range(2):
                            cf = slice(hf * (C // 2), (hf + 1) * (C // 2))
                            xA = pa.tile([P, C // 2], BF16, tag="xA", bufs=3,
                                         name="xA")
                            xB = pa.tile([P, C // 2], BF16, tag="xB", bufs=3,
                                         name="xB")
                            nc.gpsimd.dma_start(out=xA[:], in_=x_ext.ap()[rA, cf])
                            nc.gpsimd.dma_start(out=xB[:], in_=x_ext.ap()[rB, cf])
                            for ci in range(NCT // 2):
                                transpose_pair_into(
                                    xT[hf * 8 + ci][:, 2 * sp * P:(2 * sp + 2) * P],
                                    xA[:, ci * P:(ci + 1) * P],
                                    xB[:, ci * P:(ci + 1) * P],
                                )

                def x_chunk_matmuls(tch):
                    t0 = tch * TCH
                    for wTs, dstT in ((wdqT, cqT), (wdkvT, ckvT)):
                        for j in range(NNL):
                            acc = pacc.tile([P, TCH], F32, tag=f"acc{j}",
                                            name=f"acc{j}")
                            for ci in range(NCT):
                                nc.tensor.matmul(
                                    acc[:],
                                    wTs[ci][:, j * P:(j + 1) * P],
                                    xT[ci][:],
                                    start=(ci == 0),
                                    stop=(ci == NCT - 1),
                                )
                            ecopy(dstT[j][:, t0:t0 + TCH], acc[:])
                    acck = pacc.tile([RHD, TCH], F32, tag="acck", name="acck")
                    for ci in range(NCT):
                        nc.tensor.matmul(
                            acck[:],
                            wkrT[ci][:],
                            xT[ci][:],
                            start=(ci == 0),
                            stop=(ci == NCT - 1),
                        )
                    ecopy(krraw[:, t0:t0 + TCH], acck[:], pin="act")

                def wd_prep():
                    for w_ext, wTs in ((wdq_ext, wdqT), (wdkv_ext, wdkvT)):
                        for rp in range(NL // P // 2):
                            rA = slice(2 * rp * P, (2 * rp + 1) * P)
                            rB = slice((2 * rp + 1) * P, (2 * rp + 2) * P)
                            for hf in range(2):
                                cf = slice(hf * (C // 2), (hf + 1) * (C // 2))
                                sA = pa.tile([P, C // 2], BF16, tag="wsA",
                                             bufs=2, name="wsA")
                                sB = pa.tile([P, C // 2], BF16, tag="wsB",
                                             bufs=2, name="wsB")
                                nc.gpsimd.dma_start(out=sA[:], in_=w_ext.ap()[rA, cf])
                                nc.gpsimd.dma_start(out=sB[:], in_=w_ext.ap()[rB, cf])
                                for ci in range(NCT // 2):
                                    transpose_pair_into(
                                        wTs[hf * 8 + ci][:, 2 * rp * P:(2 * rp + 2) * P],
                                        sA[:, ci * P:(ci + 1) * P],
                                        sB[:, ci * P:(ci + 1) * P],
                                    )
                    kstrip = pa.tile([RHD, C], BF16, tag="kstrip", name="kstrip")
                    nc.gpsimd.dma_start(out=kstrip[:], in_=wkr_ext.ap())
                    for ci in range(NCT):
                        tp = ptp_cell[0].tile([P, 2 * P], BF16, tag="tpb",
                                              name="tp")
                        nc.tensor.transpose(
                            tp[:, :RHD], kstrip[:, ci * P:(ci + 1) * P],
                            identb[:RHD, :RHD])
                        ev, od = _deint(tp[:, :RHD])
                        nc.scalar.copy(wkrT[ci][:, 0:32], ev)
                        nc.scalar.copy(wkrT[ci][:, 32:64], od)

                def table_prep():
                    for s in range(T // P):
                        cst = pa.tile([P, RHD // 2], BF16, tag="cst", bufs=2,
                                      name="cst")
                        sst = pa.tile([P, RHD // 2], BF16, tag="sst", bufs=2,
                                      name="sst")
                        nc.gpsimd.dma_start(
                            out=cst[:], in_=cos_ext.ap()[s * P:(s + 1) * P, :])
                        nc.gpsimd.dma_start(
                            out=sst[:], in_=sin_ext.ap()[s * P:(s + 1) * P, :])
                        tp = ptp_cell[0].tile([P, 2 * P], BF16, tag="tpb",
                                              name="tp")
                        nc.tensor.transpose(tp[:32, 0:P], cst[:], identb[:])
                        nc.tensor.transpose(tp[:32, P:2 * P], sst[:], identb[:])
                        nc.vector.tensor_copy(ca4[0:32, s * P:(s + 1) * P],
                                              tp[:32, 0:P])
                        nc.vector.tensor_copy(sa4[0:32, s * P:(s + 1) * P],
                                              tp[:32, P:2 * P])
                    for d in range(1, 4):
                        nc.vector.tensor_copy(ca4[32 * d:32 * (d + 1), :],
                                              ca4[0:32, :])
                        nc.vector.tensor_copy(sa4[32 * d:32 * (d + 1), :],
                                              sa4[0:32, :])

                def wu_prep():
                    for w_ext, wT in ((wuq_ext, wuqT), (wuk_ext, wukT),
                                      (wuv_ext, wuvT)):
                        for rp in range(HLOC * HS // P // 2):
                            sA = pa.tile([P, NL], BF16, tag="usA", bufs=2,
                                         name="usA")
                            sB = pa.tile([P, NL], BF16, tag="usB", bufs=2,
                                         name="usB")
                            nc.gpsimd.dma_start(
                                out=sA[:],
                                in_=w_ext.ap()[2 * rp * P:(2 * rp + 1) * P, :])
                            nc.gpsimd.dma_start(
                                out=sB[:],
                                in_=w_ext.ap()[(2 * rp + 1) * P:(2 * rp + 2) * P, :])
                            for cs in range(NNL):
                                transpose_pair_into(
                                    wT[cs][:, 2 * rp * P:(2 * rp + 2) * P],
                                    sA[:, cs * P:(cs + 1) * P],
                                    sB[:, cs * P:(cs + 1) * P],
                                )

                def wo_wqr_prep():
                    for g in range(2):
                        strip = pa.tile([P, NL], BF16, tag="qrs", bufs=2,
                                        name="qrs")
                        nc.gpsimd.dma_start(
                            out=strip[:], in_=wqr_ext.ap()[g * P:(g + 1) * P, :])
                        for cs in range(NNL):
                            tp = ptp_cell[0].tile([P, 2 * P], BF16, tag="tpb",
                                                  name="tp")
                            nc.tensor.transpose(
                                tp[:, 0:P], strip[:, cs * P:(cs + 1) * P],
                                identb[:])
                            evA, odA = _deint(tp[:, 0:RHD])
                            evB, odB = _deint(tp[:, RHD:2 * RHD])
                            nc.scalar.copy(wqrT[g][cs][:, 0:32], evA)
                            nc.scalar.copy(wqrT[g][cs][:, 32:64], evB)
                            nc.scalar.copy(wqrT[g][cs][:, 64:96], odA)
                            nc.scalar.copy(wqrT[g][cs][:, 96:128], odB)
                    for sp in range(C // P // 2):
                        oA = pa.tile([P, HLOC * HS], BF16, tag="osA", bufs=2,
                                     name="osA")
                        oB = pa.tile([P, HLOC * HS], BF16, tag="osB", bufs=2,
                                     name="osB")
                        nc.gpsimd.dma_start(
                            out=oA[:], in_=wo_ext.ap()[2 * sp * P:(2 * sp + 1) * P, :])
                        nc.gpsimd.dma_start(
                            out=oB[:], in_=wo_ext.ap()[(2 * sp + 1) * P:(2 * sp + 2) * P, :])
                        for fs in range(HLOC):
                            transpose_pair_into(
                                woT[fs][:, 2 * sp * P:(2 * sp + 2) * P],
                                oA[:, fs * P:(fs + 1) * P],
                                oB[:, fs * P:(fs + 1) * P],
                            )

                # interleave: x transposes first so PE starts immediately,
                # weight preps slot between chunks while x DMA streams.
                x_chunk_transpose(0)
                wd_prep()
                x_chunk_matmuls(0)
                x_chunk_transpose(1)
                table_prep()
                x_chunk_matmuls(1)
                x_chunk_transpose(2)
                wu_prep()
                x_chunk_matmuls(2)
                x_chunk_transpose(3)
                wo_wqr_prep()
                x_chunk_matmuls(3)

                pa_ctx[2].__exit__(None, None, None)
                pa_ctx[1].__exit__(None, None, None)
                pa_ctx[0].__exit__(None, None, None)

                # ============ phase B: rope, up-projections, fp8 packs ======
                pb2_ctx = tc.tile_pool(name="pb2", bufs=1)
                pb2 = pb2_ctx.__enter__()
                q8 = [pb2.tile([P, 2, T], F8, tag=f"q8{h}", name=f"q8{h}")
                      for h in range(HLOC)]
                k8 = [pb2.tile([P, 2, T], F8, tag=f"k8{h}", name=f"k8{h}")
                      for h in range(HLOC)]
                vv = [pb2.tile([P, T // P, P], BF16, tag=f"vv{h}", name=f"vv{h}")
                      for h in range(HLOC)]
                for h in range(HLOC):
                    nc.vector.memset(q8[h][64:128, 1, :], 0.0)
                    nc.gpsimd.memset(k8[h][64:128, 1, :], 0.0)

                with (
                    tc.tile_pool(name="pmm", bufs=3, space="PSUM") as pmm,
                    tc.tile_pool(name="pou", bufs=2, space="PSUM") as pou,
                    tc.tile_pool(name="pde", bufs=2, space="PSUM") as pde,
                    tc.tile_pool(name="pat", bufs=1) as pat,
                ):
                    # k_r rope -> krf8 (planar re/im), shared across heads
                    krf8 = pb2.tile([RHD, T], F8, tag="krf8", name="krf8")
                    rtmp = pb2.tile([P, T], BF16, tag="rtmp", name="rtmp")
                    rro = pb2.tile([P, T], BF16, tag="rro", name="rro")
                    nc.vector.tensor_mul(rtmp[0:32, :], krraw[32:64, :], sa4[32:64, :])
                    nc.vector.tensor_mul(rtmp[32:64, :], krraw[32:64, :], ca4[32:64, :])
                    nc.vector.tensor_mul(rro[0:32, :], krraw[0:32, :], ca4[0:32, :])
                    nc.vector.tensor_mul(rro[32:64, :], krraw[0:32, :], sa4[0:32, :])
                    nc.vector.tensor_sub(krf8[0:32, :], rro[0:32, :], rtmp[0:32, :])
                    nc.vector.tensor_add(krf8[32:64, :], rro[32:64, :], rtmp[32:64, :])
                    for h in range(HLOC):
                        nc.sync.dma_start(k8[h][0:RHD, 1, :], krf8[:])

                    # up-projections, head-pair at a time
                    for g in range(2):
                        hA, hB = 2 * g, 2 * g + 1
                        qraw = pb2.tile([P, T], BF16, tag="qraw", name="qraw")
                        for ch in range(NCH):
                            sl = slice(ch * TCH, (ch + 1) * TCH)
                            for hh in (hA, hB):
                                for wT, src, dst in ((wuqT, cqT, q8),
                                                     (wukT, ckvT, k8)):
                                    acc = pmm.tile([P, TCH], F32, tag="mm",
                                                   name="mm")
                                    for nl in range(NNL):
                                        nc.tensor.matmul(
                                            acc[:],
                                            wT[nl][:, hh * P:(hh + 1) * P],
                                            src[nl][:, sl],
                                            start=(nl == 0),
                                            stop=(nl == NNL - 1),
                                        )
                                    ecopy(dst[hh][:, 0, sl], acc[:], pin="act")
                            qacc = pmm.tile([P, TCH], F32, tag="mm", name="mm")
                            for nl in range(NNL):
                                nc.tensor.matmul(
                                    qacc[:],
                                    wqrT[g][nl][:],
                                    cqT[nl][:, sl],
                                    start=(nl == 0),
                                    stop=(nl == NNL - 1),
                                )
                            ecopy(qraw[:, sl], qacc[:], pin="act")
                        # natural-layout V for both heads
                        for hh in (hA, hB):
                            for ts4 in range(T // TCH):
                                vps = pmm.tile([P, TCH], F32, tag="mm", name="mm")
                                for j in range(4):
                                    kt = ts4 * 4 + j
                                    for nl in range(NNL):
                                        nc.tensor.matmul(
                                            vps[:, j * P:(j + 1) * P],
                                            ckvT[nl][:, kt * P:(kt + 1) * P],
                                            wuvT[nl][:, hh * P:(hh + 1) * P],
                                            start=(nl == 0),
                                            stop=(nl == NNL - 1),
                                            skip_group_check=True,
                                        )
                                ecopy(
                                    vv[hh][:, ts4 * 4:(ts4 + 1) * 4, :],
                                    vps[:].rearrange("p (a b) -> p a b", a=4),
                                    pin="act",
                                )
                        # rope for the pair: rows [Are, Bre, Aim, Bim]
                        roq = pb2.tile([P, T], F8, tag="roq", bufs=2, name="roq")
                        nc.vector.tensor_mul(rtmp[0:64, :], qraw[64:128, :],
                                             sa4[64:128, :])
                        nc.vector.tensor_mul(rtmp[64:128, :], qraw[64:128, :],
                                             ca4[64:128, :])
                        nc.vector.tensor_mul(rro[0:64, :], qraw[0:64, :],
                                             ca4[0:64, :])
                        nc.vector.tensor_mul(rro[64:128, :], qraw[0:64, :],
                                             sa4[0:64, :])
                        nc.vector.tensor_sub(roq[0:64, :], rro[0:64, :],
                                             rtmp[0:64, :])
                        nc.vector.tensor_add(roq[64:128, :], rro[64:128, :],
                                             rtmp[64:128, :])
                        nc.sync.dma_start(q8[hA][0:32, 1, :], roq[0:32, :])
                        nc.sync.dma_start(q8[hA][32:64, 1, :], roq[64:96, :])
                        nc.sync.dma_start(q8[hB][0:32, 1, :], roq[32:64, :])
                        nc.sync.dma_start(q8[hB][32:64, 1, :], roq[96:128, :])

                    # ============ attention + deferred norm/output proj =====
                    deferred_b = deque()   # normalization tails
                    outproj_q = deque()    # (ohs, tq, cs_start) groups

                    def emit_outproj_group():
                        g_ohs, g_tq, cs = outproj_q.popleft()
                        g_qsl = slice(g_tq * TCH, (g_tq + 1) * TCH)
                        acc = pmm.tile([P, TCH], F32, tag="mm", name="mm")
                        for h2 in range(HLOC):
                            nc.tensor.matmul(
                                acc[:],
                                woT[h2][:, cs * P:(cs + 1) * P],
                                g_ohs[h2][:],
                                start=(h2 == 0),
                                stop=(h2 == HLOC - 1),
                            )
                        ot = pat.tile([P, TCH], F32, tag="ot", bufs=3,
                                      name="ot")
                        ecopy(ot[:], acc[:], pin="dve")
                        nc.sync.dma_start(
                            out_ext.ap()[cs * P:(cs + 1) * P, g_qsl], ot[:])

                    ohs_by_tq = {}
                    next_den = pde.tile([P, 4], F32, tag="de", name="de")
                    nc.vector.memset(next_den[:], 0.0)
                    for tq in range(NCH):
                        qsl = slice(tq * TCH, (tq + 1) * TCH)
                        ohs = []
                        ohs_by_tq[tq] = ohs
                        for h in range(HLOC):
                            outU = pou.tile([P, TCH], F32, tag="ou", name="ou")
                            den = next_den
                            nsub = (tq + 1) * 4
                            pend = deque()

                            def flush_one(outU=outU, den=den, nsub=nsub, h=h,
                                          tq=tq, pend=pend):
                                Pt, kt = pend.popleft()
                                kc, ks = kt // 4, kt % 4
                                off = ks * P if kc == tq else 0
                                for qq in range(off // P, 4):
                                    nc.tensor.matmul(
                                        den[:, qq:qq + 1],
                                        Pt[:, qq * P:(qq + 1) * P],
                                        onescol[:],
                                        start=False,
                                        stop=(kt == nsub - 1),
                                        skip_group_check=True,
                                    )
                                nc.tensor.matmul(
                                    outU[:, off:TCH],
                                    vv[h][:, kt, :],
                                    Pt[:, off:TCH],
                                    start=(kt == 0),
                                    stop=(kt == nsub - 1),
                                    skip_group_check=True,
                                )

                            for kt in range(nsub):
                                kc, ks = kt // 4, kt % 4
                                diag = kc == tq
                                off = ks * P if diag else 0
                                npr = TCH - off
                                ST = pmm.tile([P, TCH], F32, tag="mm", name="mm")
                                nc.tensor.matmul(
                                    ST[:, 0:npr],
                                    k8[h][:, :, kt * P:(kt + 1) * P],
                                    q8[h][:, :, qsl.start + off:qsl.stop],
                                    perf_mode=DR,
                                    start=True,
                                    stop=not diag,
                                )
                                if diag:
                                    # causal band mask folded in on PE:
                                    # ST[:, 0:128] += I^T @ mask
                                    nc.tensor.matmul(
                                        ST[:, 0:P],
                                        identb[:],
                                        maskbuf[:, 384:384 + P],
                                        start=False,
                                        stop=True,
                                        skip_group_check=True,
                                    )
                                Pt = pat.tile([P, TCH], BF16, tag="pt", bufs=6,
                                              name="pt")
                                nc.scalar.activation(Pt[:, off:TCH], ST[:, 0:npr],
                                                     Exp, scale=SCALE)
                                pend.append((Pt, kt))
                                if len(pend) > 2:
                                    flush_one()
                                if kt == 2 and deferred_b:
                                    # previous head's deferred normalization
                                    deferred_b.popleft()()
                                if kt >= 2 and kt % 2 == 0 and outproj_q:
                                    # one 128-row output-projection column
                                    emit_outproj_group()
                            while pend:
                                flush_one()

                            # normalization head: transpose+recip now (DVE),
                            # broadcast+apply deferred under the next head
                            den_sb = pat.tile([P, 4], F32R, tag="dsb", bufs=2,
                                              name="dsb")
                            nc.vector.tensor_copy(den_sb[:], den[:])
                            next_den = pde.tile([P, 4], F32, tag="de", name="de")
                            nc.vector.memset(next_den[:], 0.0)
                            tpd = pde.tile([4, P], F32R, tag="tpd", bufs=1,
                                           name="tpd")
                            nc.tensor.transpose(tpd[0:4, 0:P], den_sb[:], ident[:])
                            rec = pat.tile([4, P], F32, tag="rec", bufs=2,
                                           name="rec")
                            nc.vector.reciprocal(rec[:], tpd[:])
                            recb = pat.tile([4, P], BF16, tag="recb", bufs=2,
                                            name="recb")
                            nc.vector.tensor_copy(recb[:], rec[:])
                            oh = pat.tile([P, TCH], BF16, tag=f"oh{h}", bufs=2,
                                          name=f"oh{h}")
                            ohs.append(oh)

                            def norm_tail(recb=recb, outU=outU, oh=oh, h=h,
                                          tq=tq):
                                bc = pmm.tile([P, TCH], F32, tag="mm", name="mm")
                                for qq in range(4):
                                    nc.tensor.matmul(
                                        bc[:, qq * P:(qq + 1) * P],
                                        sel4[:, qq * P:(qq + 1) * P],
                                        recb[:],
                                        start=True, stop=True,
                                        skip_group_check=True,
                                    )
                                bcs = pat.tile([P, TCH], BF16, tag="bcs",
                                               bufs=2, name="bcs")
                                nc.vector.tensor_copy(bcs[:], bc[:])
                                nc.vector.tensor_mul(oh[:], outU[:], bcs[:])
                                if h == HLOC - 1:
                                    for cs0 in range(NCT):
                                        outproj_q.append(
                                            (ohs_by_tq[tq], tq, cs0))

                            deferred_b.append(norm_tail)

                    # drain deferred work
                    while deferred_b:
                        deferred_b.popleft()()
                    while outproj_q:
                        emit_outproj_group()

                pb2_ctx.__exit__(None, None, None)

    nc.compile()
    return nc


def _get_nc():
    if "nc" not in _NC_CACHE:
        _NC_CACHE["nc"] = build()
    return _NC_CACHE["nc"]


def kernel(x, freqs_cos, freqs_sin, W_dq, W_uq, W_dkv, W_uk, W_uv, W_qr, W_kr,
           W_o, trace=False, **trace_kwargs):
    nc = _get_nc()
    f32 = lambda a: np.ascontiguousarray(np.asarray(a, dtype=np.float32))
    x = f32(x); W_dq = f32(W_dq); W_uq = f32(W_uq); W_dkv = f32(W_dkv)
    W_uk = f32(W_uk); W_uv = f32(W_uv); W_qr = f32(W_qr); W_kr = f32(W_kr)
    W_o = f32(W_o)
    cos = f32(freqs_cos); sin = f32(freqs_sin)

    in_maps = []
    for c in range(8):
        b, r = divmod(c, 4)
        in_maps.append({
            "x": x[b],
            "wdq": W_dq, "wdkv": W_dkv, "wkr": W_kr,
            "wuq": W_uq[r * HLOC * HS:(r + 1) * HLOC * HS],
            "wuk": W_uk[r * HLOC * HS:(r + 1) * HLOC * HS],
            "wuv": W_uv[r * HLOC * HS:(r + 1) * HLOC * HS],
            "wqr": W_qr[r * HLOC * RHD:(r + 1) * HLOC * RHD],
            "wo": W_o[:, r * HLOC * HS:(r + 1) * HLOC * HS],
            "cos": cos, "sin": sin,
        })
    res = run_bass_kernel_spmd(nc, in_maps, core_ids=list(range(8)),
                               trace=trace, **trace_kwargs)
    out = np.zeros((B, T, C), dtype=np.float32)
    for c in range(8):
        b = c // 4
        out[b] += res.results[c]["out"].T
    kernel.last_result = res
    return out


# revision 46
# speedup vs baseline: 1.0703x; 1.0703x over previous
"""MLA-style attention kernel for 8 TRN2 NeuronCores, v3.

Sharding: core c -> batch b = c//4, heads r*4..r*4+3 where r = c%4.
The latent down-projections are REPLICATED within each 4-core batch
group (no collective, no cross-core dependency): each core computes the
full-T latents c_q/c_kv/k_r from the full x[b], then its 4 heads'
attention and a partial output projection summed on the host.

All activations stay SBUF-resident in a transposed [feature, T] layout.
Down/up-projections run in bf16 (PSUM fp32 accumulate).  Scores use
fp8e4 with DoubleRow perf mode: q/k packed as [128, 2, T] fp8 where
slot 0 holds the 128 content dims and slot 1 rows 0:64 hold the roped
rope dims (planar re/im), rows 64:128 zero.  One DoubleRow matmul per
512x128 score subtile (4x fewer PE cycles than two f32r matmuls).
Softmax denominators use N=1 ones-column matmuls accumulating into a
[128q, 4] PSUM tile (start=False onto memset zeros -- a start=True
would wipe sibling columns through the 2KB zero-region), then
transpose + reciprocal + selector broadcast matmul for normalization.
Diagonal causal blocks shrink the score matmul to the valid q-range,
memset the dead Pt columns, and add a fixed 128-wide triangular mask.
exp() pipelines two subtiles deep; the normalization tail of head h and
the output projection of chunk tq-1 are emitted under the NEXT head's
score loop so PE never drains while Act works.  V is produced directly
in natural [t, hs] layout (lhsT = ckv^T): no transposes after phase A.
Both hardware DMA queues are used: SP for W_dq/W_dkv/x/output, Act for
cos/sin, W_u/W_qr/W_o, mask, and the SBUF-to-SBUF fp8 slot copies.
"""
import math
from collections import deque
import numpy as np

import concourse.bass as bass
import concourse.bacc as bacc
import concourse.mybir as mybir
import concourse.tile as tile
from concourse.bass_utils import run_bass_kernel_spmd

F32 = mybir.dt.float32
F32R = mybir.dt.float32r
BF16 = mybir.dt.bfloat16
F8 = mybir.dt.float8e4
Exp = mybir.ActivationFunctionType.Exp
DR = mybir.MatmulPerfMode.DoubleRow

B, T, C = 2, 2048, 2048
H = 16
HS = 128
NL = 512
RHD = 64
HLOC = 4              # heads per core
P = 128
NNL = NL // P         # 4
NCT = C // P          # 16
TCH = 512
NCH = T // TCH        # 4
SCALE = 1.0 / math.sqrt(HS + RHD)
NEG = -1.0e30

_NC_CACHE = {}


def _deint(ap2d):
    # [p, 2d] -> (evens [p, d], odds [p, d]) along the free dim
    rr = ap2d.rearrange("p (d two) -> p two d", two=2)
    return rr[:, 0, :], rr[:, 1, :]


def build():
    nc = bacc.Bacc("TRN2", target_bir_lowering=False, debug=False, num_devices=8)

    x_ext = nc.dram_tensor("x", [T, C], F32R, kind="ExternalInput")
    wdq_ext = nc.dram_tensor("wdq", [NL, C], F32R, kind="ExternalInput")
    wdkv_ext = nc.dram_tensor("wdkv", [NL, C], F32R, kind="ExternalInput")
    wkr_ext = nc.dram_tensor("wkr", [RHD, C], F32R, kind="ExternalInput")
    wuq_ext = nc.dram_tensor("wuq", [HLOC * HS, NL], F32R, kind="ExternalInput")
    wuk_ext = nc.dram_tensor("wuk", [HLOC * HS, NL], F32R, kind="ExternalInput")
    wuv_ext = nc.dram_tensor("wuv", [HLOC * HS, NL], F32R, kind="ExternalInput")
    wqr_ext = nc.dram_tensor("wqr", [HLOC * RHD, NL], F32R, kind="ExternalInput")
    wo_ext = nc.dram_tensor("wo", [C, HLOC * HS], F32R, kind="ExternalInput")
    cos_ext = nc.dram_tensor("cos", [T, RHD // 2], F32R, kind="ExternalInput")
    sin_ext = nc.dram_tensor("sin", [T, RHD // 2], F32R, kind="ExternalInput")
    out_ext = nc.dram_tensor("out", [C, T], F32, kind="ExternalOutput")

    ident_dram = nc.inline_tensor(np.eye(P, dtype=np.float32), name="identc")
    # triangular mask for the 128-wide diagonal band of shrunk S^T tiles
    m = np.full((P, 896), NEG, dtype=np.float32)
    for jj in range(P):
        m[jj, 384 + jj:] = 0.0
    masks_dram = nc.inline_tensor(m, name="maskc")
    # row selector for the 1/den broadcast: sel4[k, qq*128+j] = (k == qq)
    sel = np.zeros((4, 512), dtype=np.float32)
    for qq in range(4):
        sel[qq, qq * P:(qq + 1) * P] = 1.0
    sel4_dram = nc.inline_tensor(sel, name="sel4c")

    with tile.TileContext(nc) as tc:
        with tc.tile_pool(name="pers", bufs=1) as pers:
            ptp_cell = [None]
            ident = pers.tile([P, P], F32R, tag="ident", name="ident")
            nc.sync.dma_start(ident[:], ident_dram.ap().bitcast(F32R))
            maskbuf = pers.tile([P, 896], BF16, tag="maskbuf", name="maskbuf")
            nc.gpsimd.dma_start(out=maskbuf[:], in_=masks_dram.ap())
            onescol = pers.tile([P, 1], BF16, tag="onescol", name="onescol")
            nc.vector.memset(onescol[:], 1.0)
            identb = pers.tile([P, P], BF16, tag="identb", name="identb")
            nc.vector.tensor_copy(identb[:], ident[:])
            sel4 = pers.tile([4, TCH], BF16, tag="sel4", name="sel4")
            nc.gpsimd.dma_start(out=sel4[:], in_=sel4_dram.ap())

            # rope tables, cos/sin duplicated on all four 32-row groups
            ca4 = pers.tile([P, T], BF16, tag="ca4", name="ca4")
            sa4 = pers.tile([P, T], BF16, tag="sa4", name="sa4")

            # full-T latents (bf16, [feat, T])
            cqT = [pers.tile([P, T], BF16, tag=f"cqT{i}", name=f"cqT{i}")
                   for i in range(NNL)]
            ckvT = [pers.tile([P, T], BF16, tag=f"ckvT{i}", name=f"ckvT{i}")
                    for i in range(NNL)]
            krraw = pers.tile([RHD, T], BF16, tag="krraw", name="krraw")

            _ecnt = [0]

            def ecopy(dst, src, pin=None):
                """PSUM->SBUF evacuation copy, alternating Act/DVE."""
                _ecnt[0] += 1
                eng = pin if pin else ("act" if _ecnt[0] % 2 else "dve")
                if eng == "act":
                    nc.scalar.copy(dst, src)
                else:
                    nc.vector.tensor_copy(dst, src)

            def transpose_pair_into(dst_ap, srcA, srcB, pin=None):
                bf = srcA.dtype == BF16
                tp2 = ptp_cell[0].tile([P, 2 * P], BF16 if bf else F32R,
                                       tag="tpb" if bf else "tp", name="tp")
                idn = identb if bf else ident
                nc.tensor.transpose(tp2[:, 0:P], srcA, idn[:])
                nc.tensor.transpose(tp2[:, P:2 * P], srcB, idn[:])
                ecopy(dst_ap, tp2[:], pin=pin)

            # ============ phase A: x chunks + all weight prep, interleaved ==
            with tc.tile_pool(name="pb", bufs=1) as pb:
                # -- persistent-ish weight destinations (pb outlives phase A)
                wuqT = [pb.tile([P, HLOC * HS], BF16, tag=f"wuqT{i}",
                                name=f"wuqT{i}") for i in range(NNL)]
                wukT = [pb.tile([P, HLOC * HS], BF16, tag=f"wukT{i}",
                                name=f"wukT{i}") for i in range(NNL)]
                wuvT = [pb.tile([P, HLOC * HS], BF16, tag=f"wuvT{i}",
                                name=f"wuvT{i}") for i in range(NNL)]
                wqrT = [[pb.tile([P, P], BF16, tag=f"wqrT{g}{i}",
                                 name=f"wqrT{g}{i}") for i in range(NNL)]
                        for g in range(2)]
                woT = [pb.tile([P, C], BF16, tag=f"woT{i}", name=f"woT{i}")
                       for i in range(HLOC)]

                pa_ctx = (
                    tc.tile_pool(name="pa", bufs=1),
                    tc.tile_pool(name="pacc", bufs=1, space="PSUM"),
                    tc.tile_pool(name="ptpA", bufs=3, space="PSUM"),
                )
                pa = pa_ctx[0].__enter__()
                pacc = pa_ctx[1].__enter__()
                ptp_cell[0] = pa_ctx[2].__enter__()

                wdqT = [pa.tile([P, NL], BF16, tag=f"wdqT{i}", name=f"wdqT{i}")
                        for i in range(NCT)]
                wdkvT = [pa.tile([P, NL], BF16, tag=f"wdkvT{i}",
                                 name=f"wdkvT{i}") for i in range(NCT)]
                wkrT = [pa.tile([P, RHD], BF16, tag=f"wkrT{i}", name=f"wkrT{i}")
                        for i in range(NCT)]
                xT = [pa.tile([P, TCH], BF16, tag=f"xT{i}", name=f"xT{i}")
                      for i in range(NCT)]

                def x_chunk_transpose(tch):
                    t0 = tch * TCH
                    for hf in range(2):
                        cf = slice(hf * (C // 2), (hf + 1) * (C // 2))
                        xab = pa.tile([P, 4, C // 2], BF16, tag="xab", bufs=3,
                                      name="xab")
                        nc.gpsimd.dma_start(
                            out=xab[:],
                            in_=x_ext.ap()[t0:t0 + TCH, cf].rearrange(
                                "(s p) c -> p s c", p=P))
                        for sp in range(2):
                            for ci in range(NCT // 2):
                                transpose_pair_into(
                                    xT[hf * 8 + ci][:, 2 * sp * P:(2 * sp + 2) * P],
                                    xab[:, 2 * sp, ci * P:(ci + 1) * P],
                                    xab[:, 2 * sp + 1, ci * P:(ci + 1) * P],
                                )

                def x_chunk_matmuls(tch):
                    t0 = tch * TCH
                    for wTs, dstT in ((wdqT, cqT), (wdkvT, ckvT)):
                        for j in range(NNL):
                            acc = pacc.tile([P, TCH], F32, tag=f"acc{j}",
                                            name=f"acc{j}")
                            for ci in range(NCT):
                                nc.tensor.matmul(
                                    acc[:],
                                    wTs[ci][:, j * P:(j + 1) * P],
                                    xT[ci][:],
                                    start=(ci == 0),
                                    stop=(ci == NCT - 1),
                                )
                            ecopy(dstT[j][:, t0:t0 + TCH], acc[:])
                    acck = pacc.tile([RHD, TCH], F32, tag="acck", name="acck")
                    for ci in range(NCT):
                        nc.tensor.matmul(
                            acck[:],
                            wkrT[ci][:],
                            xT[ci][:],
                            start=(ci == 0),
                            stop=(ci == NCT - 1),
                        )
                    ecopy(krraw[:, t0:t0 + TCH], acck[:], pin="act")

                def wd_prep():
                    for w_ext, wTs in ((wdq_ext, wdqT), (wdkv_ext, wdkvT)):
                        for hf in range(2):
                            cf = slice(hf * (C // 2), (hf + 1) * (C // 2))
                            sab = pa.tile([P, 4, C // 2], BF16, tag="wsab",
                                          bufs=2, name="wsab")
                            nc.gpsimd.dma_start(
                                out=sab[:],
                                in_=w_ext.ap()[:, cf].rearrange(
                                    "(s p) c -> p s c", p=P))
                            for rp in range(NL // P // 2):
                                for ci in range(NCT // 2):
                                    transpose_pair_into(
                                        wTs[hf * 8 + ci][:, 2 * rp * P:(2 * rp + 2) * P],
                                        sab[:, 2 * rp, ci * P:(ci + 1) * P],
                                        sab[:, 2 * rp + 1, ci * P:(ci + 1) * P],
                                    )
                    kstrip = pa.tile([RHD, C], BF16, tag="kstrip", name="kstrip")
                    nc.gpsimd.dma_start(out=kstrip[:], in_=wkr_ext.ap())
                    for ci in range(NCT):
                        tp = ptp_cell[0].tile([P, 2 * P], BF16, tag="tpb",
                                              name="tp")
                        nc.tensor.transpose(
                            tp[:, :RHD], kstrip[:, ci * P:(ci + 1) * P],
                            identb[:RHD, :RHD])
                        ev, od = _deint(tp[:, :RHD])
                        nc.scalar.copy(wkrT[ci][:, 0:32], ev)
                        nc.scalar.copy(wkrT[ci][:, 32:64], od)

                def table_prep():
                    cst = pa.tile([P, T // P, RHD // 2], BF16, tag="cst",
                                  name="cst")
                    sst = pa.tile([P, T // P, RHD // 2], BF16, tag="sst",
                                  name="sst")
                    nc.gpsimd.dma_start(
                        out=cst[:],
                        in_=cos_ext.ap().rearrange("(s p) f -> p s f", p=P))
                    nc.gpsimd.dma_start(
                        out=sst[:],
                        in_=sin_ext.ap().rearrange("(s p) f -> p s f", p=P))
                    for s in range(T // P):
                        tp = ptp_cell[0].tile([P, 2 * P], BF16, tag="tpb",
                                              name="tp")
                        nc.tensor.transpose(tp[:32, 0:P], cst[:, s, :], identb[:])
                        nc.tensor.transpose(tp[:32, P:2 * P], sst[:, s, :],
                                            identb[:])
                        nc.vector.tensor_copy(ca4[0:32, s * P:(s + 1) * P],
                                              tp[:32, 0:P])
                        nc.vector.tensor_copy(sa4[0:32, s * P:(s + 1) * P],
                                              tp[:32, P:2 * P])
                    for d in range(1, 4):
                        nc.vector.tensor_copy(ca4[32 * d:32 * (d + 1), :],
                                              ca4[0:32, :])
                        nc.vector.tensor_copy(sa4[32 * d:32 * (d + 1), :],
                                              sa4[0:32, :])

                def wu_prep():
                    for w_ext, wT in ((wuq_ext, wuqT), (wuk_ext, wukT),
                                      (wuv_ext, wuvT)):
                        uab = pa.tile([P, 4, NL], BF16, tag="usab", bufs=2,
                                      name="usab")
                        nc.gpsimd.dma_start(
                            out=uab[:],
                            in_=w_ext.ap().rearrange("(s p) c -> p s c", p=P))
                        for rp in range(HLOC * HS // P // 2):
                            for cs in range(NNL):
                                transpose_pair_into(
                                    wT[cs][:, 2 * rp * P:(2 * rp + 2) * P],
                                    uab[:, 2 * rp, cs * P:(cs + 1) * P],
                                    uab[:, 2 * rp + 1, cs * P:(cs + 1) * P],
                                )

                def wo_wqr_prep():
                    qab = pa.tile([P, 2, NL], BF16, tag="qrs", name="qrs")
                    nc.gpsimd.dma_start(
                        out=qab[:],
                        in_=wqr_ext.ap().rearrange("(s p) c -> p s c", p=P))
                    for g in range(2):
                        strip = qab[:, g, :]
                        for cs in range(NNL):
                            tp = ptp_cell[0].tile([P, 2 * P], BF16, tag="tpb",
                                                  name="tp")
                            nc.tensor.transpose(
                                tp[:, 0:P], strip[:, cs * P:(cs + 1) * P],
                                identb[:])
                            evA, odA = _deint(tp[:, 0:RHD])
                            evB, odB = _deint(tp[:, RHD:2 * RHD])
                            nc.scalar.copy(wqrT[g][cs][:, 0:32], evA)
                            nc.scalar.copy(wqrT[g][cs][:, 32:64], evB)
                            nc.scalar.copy(wqrT[g][cs][:, 64:96], odA)
                            nc.scalar.copy(wqrT[g][cs][:, 96:128], odB)
                    for half in range(2):
                        oab = pa.tile([P, 8, HLOC * HS], BF16, tag="osab",
                                      bufs=2, name="osab")
                        nc.gpsimd.dma_start(
                            out=oab[:],
                            in_=wo_ext.ap()[half * (C // 2):(half + 1) * (C // 2), :]
                            .rearrange("(s p) c -> p s c", p=P))
                        for sp2 in range(4):
                            sp = half * 4 + sp2
                            for fs in range(HLOC):
                                transpose_pair_into(
                                    woT[fs][:, 2 * sp * P:(2 * sp + 2) * P],
                                    oab[:, 2 * sp2, fs * P:(fs + 1) * P],
                                    oab[:, 2 * sp2 + 1, fs * P:(fs + 1) * P],
                                )

                # interleave: x transposes first so PE starts immediately,
                # weight preps slot between chunks while x DMA streams.
                x_chunk_transpose(0)
                wd_prep()
                x_chunk_matmuls(0)
                x_chunk_transpose(1)
                table_prep()
                x_chunk_matmuls(1)
                x_chunk_transpose(2)
                wu_prep()
                x_chunk_matmuls(2)
                x_chunk_transpose(3)
                wo_wqr_prep()
                x_chunk_matmuls(3)

                pa_ctx[2].__exit__(None, None, None)
                pa_ctx[1].__exit__(None, None, None)
                pa_ctx[0].__exit__(None, None, None)

                # ============ phase B: rope, up-projections, fp8 packs ======
                pb2_ctx = tc.tile_pool(name="pb2", bufs=1)
                pb2 = pb2_ctx.__enter__()
                q8 = [pb2.tile([P, 2, T], F8, tag=f"q8{h}", name=f"q8{h}")
                      for h in range(HLOC)]
                k8 = [pb2.tile([P, 2, T], F8, tag=f"k8{h}", name=f"k8{h}")
                      for h in range(HLOC)]
                vv = [pb2.tile([P, T // P, P], BF16, tag=f"vv{h}", name=f"vv{h}")
                      for h in range(HLOC)]
                for h in range(HLOC):
                    nc.vector.memset(q8[h][64:128, 1, :], 0.0)
                    nc.gpsimd.memset(k8[h][64:128, 1, :], 0.0)

                with (
                    tc.tile_pool(name="pmm", bufs=3, space="PSUM") as pmm,
                    tc.tile_pool(name="pou", bufs=2, space="PSUM") as pou,
                    tc.tile_pool(name="pde", bufs=2, space="PSUM") as pde,
                    tc.tile_pool(name="pat", bufs=1) as pat,
                ):
                    # k_r rope -> krf8 (planar re/im), shared across heads
                    krf8 = pb2.tile([RHD, T], F8, tag="krf8", name="krf8")
                    rtmp = pb2.tile([P, T], BF16, tag="rtmp", name="rtmp")
                    rro = pb2.tile([P, T], BF16, tag="rro", name="rro")
                    nc.vector.tensor_mul(rtmp[0:32, :], krraw[32:64, :], sa4[32:64, :])
                    nc.vector.tensor_mul(rtmp[32:64, :], krraw[32:64, :], ca4[32:64, :])
                    nc.vector.tensor_mul(rro[0:32, :], krraw[0:32, :], ca4[0:32, :])
                    nc.vector.tensor_mul(rro[32:64, :], krraw[0:32, :], sa4[0:32, :])
                    nc.vector.tensor_sub(krf8[0:32, :], rro[0:32, :], rtmp[0:32, :])
                    nc.vector.tensor_add(krf8[32:64, :], rro[32:64, :], rtmp[32:64, :])
                    for h in range(HLOC):
                        nc.sync.dma_start(k8[h][0:RHD, 1, :], krf8[:])

                    # up-projections, head-pair at a time
                    for g in range(2):
                        hA, hB = 2 * g, 2 * g + 1
                        qraw = pb2.tile([P, T], BF16, tag="qraw", name="qraw")
                        for ch in range(NCH):
                            sl = slice(ch * TCH, (ch + 1) * TCH)
                            for hh in (hA, hB):
                                for wT, src, dst in ((wuqT, cqT, q8),
                                                     (wukT, ckvT, k8)):
                                    acc = pmm.tile([P, TCH], F32, tag="mm",
                                                   name="mm")
                                    for nl in range(NNL):
                                        nc.tensor.matmul(
                                            acc[:],
                                            wT[nl][:, hh * P:(hh + 1) * P],
                                            src[nl][:, sl],
                                            start=(nl == 0),
                                            stop=(nl == NNL - 1),
                                        )
                                    ecopy(dst[hh][:, 0, sl], acc[:], pin="act")
                            qacc = pmm.tile([P, TCH], F32, tag="mm", name="mm")
                            for nl in range(NNL):
                                nc.tensor.matmul(
                                    qacc[:],
                                    wqrT[g][nl][:],
                                    cqT[nl][:, sl],
                                    start=(nl == 0),
                                    stop=(nl == NNL - 1),
                                )
                            ecopy(qraw[:, sl], qacc[:], pin="act")
                        # natural-layout V for both heads
                        for hh in (hA, hB):
                            for ts4 in range(T // TCH):
                                vps = pmm.tile([P, TCH], F32, tag="mm", name="mm")
                                for j in range(4):
                                    kt = ts4 * 4 + j
                                    for nl in range(NNL):
                                        nc.tensor.matmul(
                                            vps[:, j * P:(j + 1) * P],
                                            ckvT[nl][:, kt * P:(kt + 1) * P],
                                            wuvT[nl][:, hh * P:(hh + 1) * P],
                                            start=(nl == 0),
                                            stop=(nl == NNL - 1),
                                            skip_group_check=True,
                                        )
                                ecopy(
                                    vv[hh][:, ts4 * 4:(ts4 + 1) * 4, :],
                                    vps[:].rearrange("p (a b) -> p a b", a=4),
                                    pin="act",
                                )
                        # rope for the pair: rows [Are, Bre, Aim, Bim]
                        roq = pb2.tile([P, T], F8, tag="roq", bufs=2, name="roq")
                        nc.vector.tensor_mul(rtmp[0:64, :], qraw[64:128, :],
                                             sa4[64:128, :])
                        nc.vector.tensor_mul(rtmp[64:128, :], qraw[64:128, :],
                                             ca4[64:128, :])
                        nc.vector.tensor_mul(rro[0:64, :], qraw[0:64, :],
                                             ca4[0:64, :])
                        nc.vector.tensor_mul(rro[64:128, :], qraw[0:64, :],
                                             sa4[0:64, :])
                        nc.vector.tensor_sub(roq[0:64, :], rro[0:64, :],
                                             rtmp[0:64, :])
                        nc.vector.tensor_add(roq[64:128, :], rro[64:128, :],
                                             rtmp[64:128, :])
                        nc.sync.dma_start(q8[hA][0:32, 1, :], roq[0:32, :])
                        nc.sync.dma_start(q8[hA][32:64, 1, :], roq[64:96, :])
                        nc.sync.dma_start(q8[hB][0:32, 1, :], roq[32:64, :])
                        nc.sync.dma_start(q8[hB][32:64, 1, :], roq[96:128, :])

                    # ============ attention + deferred norm/output proj =====
                    deferred_b = deque()   # normalization tails
                    outproj_q = deque()    # (ohs, tq, cs_start) groups

                    def emit_outproj_group():
                        g_ohs, g_tq, cs = outproj_q.popleft()
                        g_qsl = slice(g_tq * TCH, (g_tq + 1) * TCH)
                        acc = pmm.tile([P, TCH], F32, tag="mm", name="mm")
                        for h2 in range(HLOC):
                            nc.tensor.matmul(
                                acc[:],
                                woT[h2][:, cs * P:(cs + 1) * P],
                                g_ohs[h2][:],
                                start=(h2 == 0),
                                stop=(h2 == HLOC - 1),
                            )
                        ot = pat.tile([P, TCH], F32, tag="ot", bufs=3,
                                      name="ot")
                        ecopy(ot[:], acc[:], pin="dve")
                        nc.sync.dma_start(
                            out_ext.ap()[cs * P:(cs + 1) * P, g_qsl], ot[:])

                    ohs_by_tq = {}
                    next_den = pde.tile([P, 4], F32, tag="de", name="de")
                    nc.vector.memset(next_den[:], 0.0)
                    for tq in range(NCH):
                        qsl = slice(tq * TCH, (tq + 1) * TCH)
                        ohs = []
                        ohs_by_tq[tq] = ohs
                        for h in range(HLOC):
                            outU = pou.tile([P, TCH], F32, tag="ou", name="ou")
                            den = next_den
                            nsub = (tq + 1) * 4
                            pend = deque()

                            def flush_one(outU=outU, den=den, nsub=nsub, h=h,
                                          tq=tq, pend=pend):
                                Pt, kt = pend.popleft()
                                kc, ks = kt // 4, kt % 4
                                off = ks * P if kc == tq else 0
                                for qq in range(off // P, 4):
                                    nc.tensor.matmul(
                                        den[:, qq:qq + 1],
                                        Pt[:, qq * P:(qq + 1) * P],
                                        onescol[:],
                                        start=False,
                                        stop=(kt == nsub - 1),
                                        skip_group_check=True,
                                    )
                                nc.tensor.matmul(
                                    outU[:, off:TCH],
                                    vv[h][:, kt, :],
                                    Pt[:, off:TCH],
                                    start=(kt == 0),
                                    stop=(kt == nsub - 1),
                                    skip_group_check=True,
                                )

                            for kt in range(nsub):
                                kc, ks = kt // 4, kt % 4
                                diag = kc == tq
                                off = ks * P if diag else 0
                                npr = TCH - off
                                ST = pmm.tile([P, TCH], F32, tag="mm", name="mm")
                                nc.tensor.matmul(
                                    ST[:, 0:npr],
                                    k8[h][:, :, kt * P:(kt + 1) * P],
                                    q8[h][:, :, qsl.start + off:qsl.stop],
                                    perf_mode=DR,
                                    start=True,
                                    stop=not diag,
                                )
                                if diag:
                                    # causal band mask folded in on PE:
                                    # ST[:, 0:128] += I^T @ mask
                                    nc.tensor.matmul(
                                        ST[:, 0:P],
                                        identb[:],
                                        maskbuf[:, 384:384 + P],
                                        start=False,
                                        stop=True,
                                        skip_group_check=True,
                                    )
                                Pt = pat.tile([P, TCH], BF16, tag="pt", bufs=6,
                                              name="pt")
                                nc.scalar.activation(Pt[:, off:TCH], ST[:, 0:npr],
                                                     Exp, scale=SCALE)
                                pend.append((Pt, kt))
                                if len(pend) > 2:
                                    flush_one()
                                if kt == 2 and deferred_b:
                                    # previous head's deferred normalization
                                    deferred_b.popleft()()
                                if kt >= 2 and kt % 2 == 0 and outproj_q:
                                    # one 128-row output-projection column
                                    emit_outproj_group()
                            while pend:
                                flush_one()

                            # normalization head: transpose+recip now (DVE),
                            # broadcast+apply deferred under the next head
                            den_sb = pat.tile([P, 4], F32R, tag="dsb", bufs=2,
                                              name="dsb")
                            nc.vector.tensor_copy(den_sb[:], den[:])
                            next_den = pde.tile([P, 4], F32, tag="de", name="de")
                            nc.vector.memset(next_den[:], 0.0)
                            tpd = pde.tile([4, P], F32R, tag="tpd", bufs=1,
                                           name="tpd")
                            nc.tensor.transpose(tpd[0:4, 0:P], den_sb[:], ident[:])
                            rec = pat.tile([4, P], F32, tag="rec", bufs=2,
                                           name="rec")
                            nc.vector.reciprocal(rec[:], tpd[:])
                            recb = pat.tile([4, P], BF16, tag="recb", bufs=2,
                                            name="recb")
                            nc.vector.tensor_copy(recb[:], rec[:])
                            oh = pat.tile([P, TCH], BF16, tag=f"oh{h}", bufs=2,
                                          name=f"oh{h}")
                            ohs.append(oh)

                            def norm_tail(recb=recb, outU=outU, oh=oh, h=h,
                                          tq=tq):
                                bc = pmm.tile([P, TCH], F32, tag="mm", name="mm")
                                for qq in range(4):
                                    nc.tensor.matmul(
                                        bc[:, qq * P:(qq + 1) * P],
                                        sel4[:, qq * P:(qq + 1) * P],
                                        recb[:],
                                        start=True, stop=True,
                                        skip_group_check=True,
                                    )
                                bcs = pat.tile([P, TCH], BF16, tag="bcs",
                                               bufs=2, name="bcs")
                                nc.vector.tensor_copy(bcs[:], bc[:])
                                nc.vector.tensor_mul(oh[:], outU[:], bcs[:])
                                if h == HLOC - 1:
                                    for cs0 in range(NCT):
                                        outproj_q.append(
                                            (ohs_by_tq[tq], tq, cs0))

                            deferred_b.append(norm_tail)

                    # drain deferred work
                    while deferred_b:
                        deferred_b.popleft()()
                    while outproj_q:
                        emit_outproj_group()

                pb2_ctx.__exit__(None, None, None)

    nc.compile()
    return nc


def _get_nc():
    if "nc" not in _NC_CACHE:
        _NC_CACHE["nc"] = build()
    return _NC_CACHE["nc"]


def kernel(x, freqs_cos, freqs_sin, W_dq, W_uq, W_dkv, W_uk, W_uv, W_qr, W_kr,
           W_o, trace=False, **trace_kwargs):
    nc = _get_nc()
    f32 = lambda a: np.ascontiguousarray(np.asarray(a, dtype=np.float32))
    x = f32(x); W_dq = f32(W_dq); W_uq = f32(W_uq); W_dkv = f32(W_dkv)
    W_uk = f32(W_uk); W_uv = f32(W_uv); W_qr = f32(W_qr); W_kr = f32(W_kr)
    W_o = f32(W_o)
    cos = f32(freqs_cos); sin = f32(freqs_sin)

    in_maps = []
    for c in range(8):
        b, r = divmod(c, 4)
        in_maps.append({
            "x": x[b],
            "wdq": W_dq, "wdkv": W_dkv, "wkr": W_kr,
            "wuq": W_uq[r * HLOC * HS:(r + 1) * HLOC * HS],
            "wuk": W_uk[r * HLOC * HS:(r + 1) * HLOC * HS],
            "wuv": W_uv[r * HLOC * HS:(r + 1) * HLOC * HS],
            "wqr": W_qr[r * HLOC * RHD:(r + 1) * HLOC * RHD],
            "wo": W_o[:, r * HLOC * HS:(r + 1) * HLOC * HS],
            "cos": cos, "sin": sin,
        })
    res = run_bass_kernel_spmd(nc, in_maps, core_ids=list(range(8)),
                               trace=trace, **trace_kwargs)
    out = np.zeros((B, T, C), dtype=np.float32)
    for c in range(8):
        b = c // 4
        out[b] += res.results[c]["out"].T
    kernel.last_result = res
    return out


# revision 47
# speedup vs baseline: 1.0714x; 1.0010x over previous
"""MLA-style attention kernel for 8 TRN2 NeuronCores, v3.

Sharding: core c -> batch b = c//4, heads r*4..r*4+3 where r = c%4.
The latent down-projections are REPLICATED within each 4-core batch
group (no collective, no cross-core dependency): each core computes the
full-T latents c_q/c_kv/k_r from the full x[b], then its 4 heads'
attention and a partial output projection summed on the host.

All activations stay SBUF-resident in a transposed [feature, T] layout.
Down/up-projections run in bf16 (PSUM fp32 accumulate).  Scores use
fp8e4 with DoubleRow perf mode: q/k packed as [128, 2, T] fp8 where
slot 0 holds the 128 content dims and slot 1 rows 0:64 hold the roped
rope dims (planar re/im), rows 64:128 zero.  One DoubleRow matmul per
512x128 score subtile (4x fewer PE cycles than two f32r matmuls).
Softmax denominators use N=1 ones-column matmuls accumulating into a
[128q, 4] PSUM tile (start=False onto memset zeros -- a start=True
would wipe sibling columns through the 2KB zero-region), then
transpose + reciprocal + selector broadcast matmul for normalization.
Diagonal causal blocks shrink the score matmul to the valid q-range,
memset the dead Pt columns, and add a fixed 128-wide triangular mask.
exp() pipelines two subtiles deep; the normalization tail of head h and
the output projection of chunk tq-1 are emitted under the NEXT head's
score loop so PE never drains while Act works.  V is produced directly
in natural [t, hs] layout (lhsT = ckv^T): no transposes after phase A.
Both hardware DMA queues are used: SP for W_dq/W_dkv/x/output, Act for
cos/sin, W_u/W_qr/W_o, mask, and the SBUF-to-SBUF fp8 slot copies.
"""
import math
from collections import deque
import numpy as np

import concourse.bass as bass
import concourse.bacc as bacc
import concourse.mybir as mybir
import concourse.tile as tile
from concourse.bass_utils import run_bass_kernel_spmd

F32 = mybir.dt.float32
F32R = mybir.dt.float32r
BF16 = mybir.dt.bfloat16
F8 = mybir.dt.float8e4
Exp = mybir.ActivationFunctionType.Exp
DR = mybir.MatmulPerfMode.DoubleRow

B, T, C = 2, 2048, 2048
H = 16
HS = 128
NL = 512
RHD = 64
HLOC = 4              # heads per core
P = 128
NNL = NL // P         # 4
NCT = C // P          # 16
TCH = 512
NCH = T // TCH        # 4
SCALE = 1.0 / math.sqrt(HS + RHD)
NEG = -1.0e30

_NC_CACHE = {}


def _deint(ap2d):
    # [p, 2d] -> (evens [p, d], odds [p, d]) along the free dim
    rr = ap2d.rearrange("p (d two) -> p two d", two=2)
    return rr[:, 0, :], rr[:, 1, :]


def build():
    nc = bacc.Bacc("TRN2", target_bir_lowering=False, debug=False, num_devices=8)

    x_ext = nc.dram_tensor("x", [T, C], F32R, kind="ExternalInput")
    wdq_ext = nc.dram_tensor("wdq", [NL, C], F32R, kind="ExternalInput")
    wdkv_ext = nc.dram_tensor("wdkv", [NL, C], F32R, kind="ExternalInput")
    wkr_ext = nc.dram_tensor("wkr", [RHD, C], F32R, kind="ExternalInput")
    wuq_ext = nc.dram_tensor("wuq", [HLOC * HS, NL], F32R, kind="ExternalInput")
    wuk_ext = nc.dram_tensor("wuk", [HLOC * HS, NL], F32R, kind="ExternalInput")
    wuv_ext = nc.dram_tensor("wuv", [HLOC * HS, NL], F32R, kind="ExternalInput")
    wqr_ext = nc.dram_tensor("wqr", [HLOC * RHD, NL], F32R, kind="ExternalInput")
    wo_ext = nc.dram_tensor("wo", [C, HLOC * HS], F32R, kind="ExternalInput")
    cos_ext = nc.dram_tensor("cos", [T, RHD // 2], F32R, kind="ExternalInput")
    sin_ext = nc.dram_tensor("sin", [T, RHD // 2], F32R, kind="ExternalInput")
    out_ext = nc.dram_tensor("out", [C, T], F32, kind="ExternalOutput")

    ident_dram = nc.inline_tensor(np.eye(P, dtype=np.float32), name="identc")
    # triangular mask for the 128-wide diagonal band of shrunk S^T tiles
    m = np.full((P, 896), NEG, dtype=np.float32)
    for jj in range(P):
        m[jj, 384 + jj:] = 0.0
    masks_dram = nc.inline_tensor(m, name="maskc")
    # row selector for the 1/den broadcast: sel4[k, qq*128+j] = (k == qq)
    sel = np.zeros((4, 512), dtype=np.float32)
    for qq in range(4):
        sel[qq, qq * P:(qq + 1) * P] = 1.0
    sel4_dram = nc.inline_tensor(sel, name="sel4c")

    with tile.TileContext(nc) as tc:
        with tc.tile_pool(name="pers", bufs=1) as pers:
            ptp_cell = [None]
            ident = pers.tile([P, P], F32R, tag="ident", name="ident")
            nc.sync.dma_start(ident[:], ident_dram.ap().bitcast(F32R))
            maskbuf = pers.tile([P, 896], BF16, tag="maskbuf", name="maskbuf")
            nc.gpsimd.dma_start(out=maskbuf[:], in_=masks_dram.ap())
            onescol = pers.tile([P, 1], BF16, tag="onescol", name="onescol")
            nc.vector.memset(onescol[:], 1.0)
            identb = pers.tile([P, P], BF16, tag="identb", name="identb")
            nc.vector.tensor_copy(identb[:], ident[:])
            sel4 = pers.tile([4, TCH], BF16, tag="sel4", name="sel4")
            nc.gpsimd.dma_start(out=sel4[:], in_=sel4_dram.ap())

            # rope tables, cos/sin duplicated on all four 32-row groups
            ca4 = pers.tile([P, T], BF16, tag="ca4", name="ca4")
            sa4 = pers.tile([P, T], BF16, tag="sa4", name="sa4")

            # full-T latents (bf16, [feat, T])
            cqT = [pers.tile([P, T], BF16, tag=f"cqT{i}", name=f"cqT{i}")
                   for i in range(NNL)]
            ckvT = [pers.tile([P, T], BF16, tag=f"ckvT{i}", name=f"ckvT{i}")
                    for i in range(NNL)]
            krraw = pers.tile([RHD, T], BF16, tag="krraw", name="krraw")

            _ecnt = [0]

            def ecopy(dst, src, pin=None):
                """PSUM->SBUF evacuation copy, alternating Act/DVE."""
                _ecnt[0] += 1
                eng = pin if pin else ("act" if _ecnt[0] % 2 else "dve")
                if eng == "act":
                    nc.scalar.copy(dst, src)
                else:
                    nc.vector.tensor_copy(dst, src)

            def transpose_pair_into(dst_ap, srcA, srcB, pin=None):
                bf = srcA.dtype == BF16
                tp2 = ptp_cell[0].tile([P, 2 * P], BF16 if bf else F32R,
                                       tag="tpb" if bf else "tp", name="tp")
                idn = identb if bf else ident
                nc.tensor.transpose(tp2[:, 0:P], srcA, idn[:])
                nc.tensor.transpose(tp2[:, P:2 * P], srcB, idn[:])
                ecopy(dst_ap, tp2[:], pin=pin)

            # ============ phase A: x chunks + all weight prep, interleaved ==
            with tc.tile_pool(name="pb", bufs=1) as pb:
                # -- persistent-ish weight destinations (pb outlives phase A)
                wuqT = [pb.tile([P, HLOC * HS], BF16, tag=f"wuqT{i}",
                                name=f"wuqT{i}") for i in range(NNL)]
                wukT = [pb.tile([P, HLOC * HS], BF16, tag=f"wukT{i}",
                                name=f"wukT{i}") for i in range(NNL)]
                wuvT = [pb.tile([P, HLOC * HS], BF16, tag=f"wuvT{i}",
                                name=f"wuvT{i}") for i in range(NNL)]
                wqrT = [[pb.tile([P, P], BF16, tag=f"wqrT{g}{i}",
                                 name=f"wqrT{g}{i}") for i in range(NNL)]
                        for g in range(2)]
                woT = [pb.tile([P, C], BF16, tag=f"woT{i}", name=f"woT{i}")
                       for i in range(HLOC)]

                pa_ctx = (
                    tc.tile_pool(name="pa", bufs=1),
                    tc.tile_pool(name="pacc", bufs=1, space="PSUM"),
                    tc.tile_pool(name="ptpA", bufs=3, space="PSUM"),
                )
                pa = pa_ctx[0].__enter__()
                pacc = pa_ctx[1].__enter__()
                ptp_cell[0] = pa_ctx[2].__enter__()

                wdqT = [pa.tile([P, NL], BF16, tag=f"wdqT{i}", name=f"wdqT{i}")
                        for i in range(NCT)]
                wdkvT = [pa.tile([P, NL], BF16, tag=f"wdkvT{i}",
                                 name=f"wdkvT{i}") for i in range(NCT)]
                wkrT = [pa.tile([P, RHD], BF16, tag=f"wkrT{i}", name=f"wkrT{i}")
                        for i in range(NCT)]
                xT = [pa.tile([P, TCH], BF16, tag=f"xT{i}", name=f"xT{i}")
                      for i in range(NCT)]

                def x_chunk_transpose(tch):
                    t0 = tch * TCH
                    for hf in range(2):
                        cf = slice(hf * (C // 2), (hf + 1) * (C // 2))
                        xab = pa.tile([P, 4, C // 2], BF16, tag="xab", bufs=3,
                                      name="xab")
                        nc.gpsimd.dma_start(
                            out=xab[:],
                            in_=x_ext.ap()[t0:t0 + TCH, cf].rearrange(
                                "(s p) c -> p s c", p=P))
                        for sp in range(2):
                            for ci in range(NCT // 2):
                                transpose_pair_into(
                                    xT[hf * 8 + ci][:, 2 * sp * P:(2 * sp + 2) * P],
                                    xab[:, 2 * sp, ci * P:(ci + 1) * P],
                                    xab[:, 2 * sp + 1, ci * P:(ci + 1) * P],
                                )

                def x_chunk_matmuls(tch):
                    t0 = tch * TCH
                    for wTs, dstT in ((wdqT, cqT), (wdkvT, ckvT)):
                        for j in range(NNL):
                            acc = pacc.tile([P, TCH], F32, tag=f"acc{j}",
                                            name=f"acc{j}")
                            for ci in range(NCT):
                                nc.tensor.matmul(
                                    acc[:],
                                    wTs[ci][:, j * P:(j + 1) * P],
                                    xT[ci][:],
                                    start=(ci == 0),
                                    stop=(ci == NCT - 1),
                                )
                            ecopy(dstT[j][:, t0:t0 + TCH], acc[:])
                    acck = pacc.tile([RHD, TCH], F32, tag="acck", name="acck")
                    for ci in range(NCT):
                        nc.tensor.matmul(
                            acck[:],
                            wkrT[ci][:],
                            xT[ci][:],
                            start=(ci == 0),
                            stop=(ci == NCT - 1),
                        )
                    ecopy(krraw[:, t0:t0 + TCH], acck[:], pin="act")

                def wd_prep():
                    for w_ext, wTs in ((wdq_ext, wdqT), (wdkv_ext, wdkvT)):
                        for hf in range(2):
                            cf = slice(hf * (C // 2), (hf + 1) * (C // 2))
                            sab = pa.tile([P, 4, C // 2], BF16, tag="wsab",
                                          bufs=2, name="wsab")
                            nc.gpsimd.dma_start(
                                out=sab[:],
                                in_=w_ext.ap()[:, cf].rearrange(
                                    "(s p) c -> p s c", p=P))
                            for rp in range(NL // P // 2):
                                for ci in range(NCT // 2):
                                    transpose_pair_into(
                                        wTs[hf * 8 + ci][:, 2 * rp * P:(2 * rp + 2) * P],
                                        sab[:, 2 * rp, ci * P:(ci + 1) * P],
                                        sab[:, 2 * rp + 1, ci * P:(ci + 1) * P],
                                    )
                    kstrip = pa.tile([RHD, C], BF16, tag="kstrip", name="kstrip")
                    nc.gpsimd.dma_start(out=kstrip[:], in_=wkr_ext.ap())
                    for ci in range(NCT):
                        tp = ptp_cell[0].tile([P, 2 * P], BF16, tag="tpb",
                                              name="tp")
                        nc.tensor.transpose(
                            tp[:, :RHD], kstrip[:, ci * P:(ci + 1) * P],
                            identb[:RHD, :RHD])
                        ev, od = _deint(tp[:, :RHD])
                        nc.scalar.copy(wkrT[ci][:, 0:32], ev)
                        nc.scalar.copy(wkrT[ci][:, 32:64], od)

                def table_prep():
                    cst = pa.tile([P, T // P, RHD // 2], BF16, tag="cst",
                                  name="cst")
                    sst = pa.tile([P, T // P, RHD // 2], BF16, tag="sst",
                                  name="sst")
                    nc.gpsimd.dma_start(
                        out=cst[:],
                        in_=cos_ext.ap().rearrange("(s p) f -> p s f", p=P))
                    nc.gpsimd.dma_start(
                        out=sst[:],
                        in_=sin_ext.ap().rearrange("(s p) f -> p s f", p=P))
                    for s in range(T // P):
                        tp = ptp_cell[0].tile([P, 2 * P], BF16, tag="tpb",
                                              name="tp")
                        nc.tensor.transpose(tp[:32, 0:P], cst[:, s, :], identb[:])
                        nc.tensor.transpose(tp[:32, P:2 * P], sst[:, s, :],
                                            identb[:])
                        nc.vector.tensor_copy(ca4[0:32, s * P:(s + 1) * P],
                                              tp[:32, 0:P])
                        nc.vector.tensor_copy(sa4[0:32, s * P:(s + 1) * P],
                                              tp[:32, P:2 * P])
                    for d in range(1, 4):
                        nc.vector.tensor_copy(ca4[32 * d:32 * (d + 1), :],
                                              ca4[0:32, :])
                        nc.vector.tensor_copy(sa4[32 * d:32 * (d + 1), :],
                                              sa4[0:32, :])

                def wu_prep():
                    for w_ext, wT in ((wuq_ext, wuqT), (wuk_ext, wukT),
                                      (wuv_ext, wuvT)):
                        uab = pa.tile([P, 4, NL], BF16, tag="usab", bufs=2,
                                      name="usab")
                        nc.gpsimd.dma_start(
                            out=uab[:],
                            in_=w_ext.ap().rearrange("(s p) c -> p s c", p=P))
                        for rp in range(HLOC * HS // P // 2):
                            for cs in range(NNL):
                                transpose_pair_into(
                                    wT[cs][:, 2 * rp * P:(2 * rp + 2) * P],
                                    uab[:, 2 * rp, cs * P:(cs + 1) * P],
                                    uab[:, 2 * rp + 1, cs * P:(cs + 1) * P],
                                )

                def wo_wqr_prep():
                    qab = pa.tile([P, 2, NL], BF16, tag="qrs", name="qrs")
                    nc.gpsimd.dma_start(
                        out=qab[:],
                        in_=wqr_ext.ap().rearrange("(s p) c -> p s c", p=P))
                    for g in range(2):
                        strip = qab[:, g, :]
                        for cs in range(NNL):
                            tp = ptp_cell[0].tile([P, 2 * P], BF16, tag="tpb",
                                                  name="tp")
                            nc.tensor.transpose(
                                tp[:, 0:P], strip[:, cs * P:(cs + 1) * P],
                                identb[:])
                            evA, odA = _deint(tp[:, 0:RHD])
                            evB, odB = _deint(tp[:, RHD:2 * RHD])
                            nc.scalar.copy(wqrT[g][cs][:, 0:32], evA)
                            nc.scalar.copy(wqrT[g][cs][:, 32:64], evB)
                            nc.scalar.copy(wqrT[g][cs][:, 64:96], odA)
                            nc.scalar.copy(wqrT[g][cs][:, 96:128], odB)
                    for half in range(2):
                        oab = pa.tile([P, 8, HLOC * HS], BF16, tag="osab",
                                      bufs=2, name="osab")
                        nc.gpsimd.dma_start(
                            out=oab[:],
                            in_=wo_ext.ap()[half * (C // 2):(half + 1) * (C // 2), :]
                            .rearrange("(s p) c -> p s c", p=P))
                        for sp2 in range(4):
                            sp = half * 4 + sp2
                            for fs in range(HLOC):
                                transpose_pair_into(
                                    woT[fs][:, 2 * sp * P:(2 * sp + 2) * P],
                                    oab[:, 2 * sp2, fs * P:(fs + 1) * P],
                                    oab[:, 2 * sp2 + 1, fs * P:(fs + 1) * P],
                                )

                # interleave: x transposes first so PE starts immediately,
                # weight preps slot between chunks while x DMA streams.
                x_chunk_transpose(0)
                wd_prep()
                x_chunk_matmuls(0)
                x_chunk_transpose(1)
                table_prep()
                x_chunk_matmuls(1)
                x_chunk_transpose(2)
                wu_prep()
                x_chunk_matmuls(2)
                x_chunk_transpose(3)
                wo_wqr_prep()
                x_chunk_matmuls(3)

                pa_ctx[2].__exit__(None, None, None)
                pa_ctx[1].__exit__(None, None, None)
                pa_ctx[0].__exit__(None, None, None)

                # ============ phase B: rope, up-projections, fp8 packs ======
                pb2_ctx = tc.tile_pool(name="pb2", bufs=1)
                pb2 = pb2_ctx.__enter__()
                q8 = [pb2.tile([P, 2, T], F8, tag=f"q8{h}", name=f"q8{h}")
                      for h in range(HLOC)]
                k8 = [pb2.tile([P, 2, T], F8, tag=f"k8{h}", name=f"k8{h}")
                      for h in range(HLOC)]
                vv = [pb2.tile([P, T // P, P], BF16, tag=f"vv{h}", name=f"vv{h}")
                      for h in range(HLOC)]
                for h in range(HLOC):
                    nc.vector.memset(q8[h][64:128, 1, :], 0.0)
                    nc.gpsimd.memset(k8[h][64:128, 1, :], 0.0)

                with (
                    tc.tile_pool(name="pmm", bufs=3, space="PSUM") as pmm,
                    tc.tile_pool(name="pou", bufs=2, space="PSUM") as pou,
                    tc.tile_pool(name="pde", bufs=2, space="PSUM") as pde,
                    tc.tile_pool(name="pat", bufs=1) as pat,
                ):
                    # k_r rope -> krf8 (planar re/im), shared across heads
                    krf8 = pb2.tile([RHD, T], F8, tag="krf8", name="krf8")
                    rtmp = pb2.tile([P, T], BF16, tag="rtmp", name="rtmp")
                    rro = pb2.tile([P, T], BF16, tag="rro", name="rro")
                    nc.vector.tensor_mul(rtmp[0:32, :], krraw[32:64, :], sa4[32:64, :])
                    nc.vector.tensor_mul(rtmp[32:64, :], krraw[32:64, :], ca4[32:64, :])
                    nc.vector.tensor_mul(rro[0:32, :], krraw[0:32, :], ca4[0:32, :])
                    nc.vector.tensor_mul(rro[32:64, :], krraw[0:32, :], sa4[0:32, :])
                    nc.vector.tensor_sub(krf8[0:32, :], rro[0:32, :], rtmp[0:32, :])
                    nc.vector.tensor_add(krf8[32:64, :], rro[32:64, :], rtmp[32:64, :])
                    for h in range(HLOC):
                        nc.sync.dma_start(k8[h][0:RHD, 1, :], krf8[:])

                    # up-projections, head-pair at a time
                    for g in range(2):
                        hA, hB = 2 * g, 2 * g + 1
                        qraw = pb2.tile([P, T], BF16, tag="qraw", name="qraw")
                        for ch in range(NCH):
                            sl = slice(ch * TCH, (ch + 1) * TCH)
                            for hh in (hA, hB):
                                for wT, src, dst in ((wuqT, cqT, q8),
                                                     (wukT, ckvT, k8)):
                                    acc = pmm.tile([P, TCH], F32, tag="mm",
                                                   name="mm")
                                    for nl in range(NNL):
                                        nc.tensor.matmul(
                                            acc[:],
                                            wT[nl][:, hh * P:(hh + 1) * P],
                                            src[nl][:, sl],
                                            start=(nl == 0),
                                            stop=(nl == NNL - 1),
                                        )
                                    ecopy(dst[hh][:, 0, sl], acc[:], pin="act")
                            qacc = pmm.tile([P, TCH], F32, tag="mm", name="mm")
                            for nl in range(NNL):
                                nc.tensor.matmul(
                                    qacc[:],
                                    wqrT[g][nl][:],
                                    cqT[nl][:, sl],
                                    start=(nl == 0),
                                    stop=(nl == NNL - 1),
                                )
                            ecopy(qraw[:, sl], qacc[:], pin="act")
                        # rope for the pair: rows [Are, Bre, Aim, Bim]
                        roq = pb2.tile([P, T], F8, tag="roq", bufs=2, name="roq")
                        nc.vector.tensor_mul(rtmp[0:64, :], qraw[64:128, :],
                                             sa4[64:128, :])
                        nc.vector.tensor_mul(rtmp[64:128, :], qraw[64:128, :],
                                             ca4[64:128, :])
                        nc.vector.tensor_mul(rro[0:64, :], qraw[0:64, :],
                                             ca4[0:64, :])
                        nc.vector.tensor_mul(rro[64:128, :], qraw[0:64, :],
                                             sa4[0:64, :])
                        nc.vector.tensor_sub(roq[0:64, :], rro[0:64, :],
                                             rtmp[0:64, :])
                        nc.vector.tensor_add(roq[64:128, :], rro[64:128, :],
                                             rtmp[64:128, :])
                        nc.sync.dma_start(q8[hA][0:32, 1, :], roq[0:32, :])
                        nc.sync.dma_start(q8[hA][32:64, 1, :], roq[64:96, :])
                        nc.sync.dma_start(q8[hB][0:32, 1, :], roq[32:64, :])
                        nc.sync.dma_start(q8[hB][32:64, 1, :], roq[96:128, :])
                        # natural-layout V for both heads
                        for hh in (hA, hB):
                            for ts4 in range(T // TCH):
                                vps = pmm.tile([P, TCH], F32, tag="mm", name="mm")
                                for j in range(4):
                                    kt = ts4 * 4 + j
                                    for nl in range(NNL):
                                        nc.tensor.matmul(
                                            vps[:, j * P:(j + 1) * P],
                                            ckvT[nl][:, kt * P:(kt + 1) * P],
                                            wuvT[nl][:, hh * P:(hh + 1) * P],
                                            start=(nl == 0),
                                            stop=(nl == NNL - 1),
                                            skip_group_check=True,
                                        )
                                ecopy(
                                    vv[hh][:, ts4 * 4:(ts4 + 1) * 4, :],
                                    vps[:].rearrange("p (a b) -> p a b", a=4),
                                    pin="act",
                                )

                    # ============ attention + deferred norm/output proj =====
                    deferred_b = deque()   # normalization tails
                    outproj_q = deque()    # (ohs, tq, cs_start) groups

                    def emit_outproj_group():
                        g_ohs, g_tq, cs = outproj_q.popleft()
                        g_qsl = slice(g_tq * TCH, (g_tq + 1) * TCH)
                        acc = pmm.tile([P, TCH], F32, tag="mm", name="mm")
                        for h2 in range(HLOC):
                            nc.tensor.matmul(
                                acc[:],
                                woT[h2][:, cs * P:(cs + 1) * P],
                                g_ohs[h2][:],
                                start=(h2 == 0),
                                stop=(h2 == HLOC - 1),
                            )
                        ot = pat.tile([P, TCH], F32, tag="ot", bufs=3,
                                      name="ot")
                        ecopy(ot[:], acc[:], pin="dve")
                        nc.sync.dma_start(
                            out_ext.ap()[cs * P:(cs + 1) * P, g_qsl], ot[:])

                    ohs_by_tq = {}
                    next_den = pde.tile([P, 4], F32, tag="de", name="de")
                    nc.vector.memset(next_den[:], 0.0)
                    for tq in range(NCH):
                        qsl = slice(tq * TCH, (tq + 1) * TCH)
                        ohs = []
                        ohs_by_tq[tq] = ohs
                        for h in range(HLOC):
                            outU = pou.tile([P, TCH], F32, tag="ou", name="ou")
                            den = next_den
                            nsub = (tq + 1) * 4
                            pend = deque()

                            def flush_one(outU=outU, den=den, nsub=nsub, h=h,
                                          tq=tq, pend=pend):
                                Pt, kt = pend.popleft()
                                kc, ks = kt // 4, kt % 4
                                off = ks * P if kc == tq else 0
                                for qq in range(off // P, 4):
                                    nc.tensor.matmul(
                                        den[:, qq:qq + 1],
                                        Pt[:, qq * P:(qq + 1) * P],
                                        onescol[:],
                                        start=False,
                                        stop=(kt == nsub - 1),
                                        skip_group_check=True,
                                    )
                                nc.tensor.matmul(
                                    outU[:, off:TCH],
                                    vv[h][:, kt, :],
                                    Pt[:, off:TCH],
                                    start=(kt == 0),
                                    stop=(kt == nsub - 1),
                                    skip_group_check=True,
                                )

                            for kt in range(nsub):
                                kc, ks = kt // 4, kt % 4
                                diag = kc == tq
                                off = ks * P if diag else 0
                                npr = TCH - off
                                ST = pmm.tile([P, TCH], F32, tag="mm", name="mm")
                                nc.tensor.matmul(
                                    ST[:, 0:npr],
                                    k8[h][:, :, kt * P:(kt + 1) * P],
                                    q8[h][:, :, qsl.start + off:qsl.stop],
                                    perf_mode=DR,
                                    start=True,
                                    stop=not diag,
                                )
                                if diag:
                                    # causal band mask folded in on PE:
                                    # ST[:, 0:128] += I^T @ mask
                                    nc.tensor.matmul(
                                        ST[:, 0:P],
                                        identb[:],
                                        maskbuf[:, 384:384 + P],
                                        start=False,
                                        stop=True,
                                        skip_group_check=True,
                                    )
                                Pt = pat.tile([P, TCH], BF16, tag="pt", bufs=8,
                                              name="pt")
                                nc.scalar.activation(Pt[:, off:TCH], ST[:, 0:npr],
                                                     Exp, scale=SCALE)
                                pend.append((Pt, kt))
                                if len(pend) > 3:
                                    flush_one()
                                if kt == 2 and deferred_b:
                                    # previous head's deferred normalization
                                    deferred_b.popleft()()
                                if kt >= 2 and kt % 2 == 0 and outproj_q:
                                    # one 128-row output-projection column
                                    emit_outproj_group()
                            while pend:
                                flush_one()

                            # normalization head: transpose+recip now (DVE),
                            # broadcast+apply deferred under the next head
                            den_sb = pat.tile([P, 4], F32R, tag="dsb", bufs=2,
                                              name="dsb")
                            nc.vector.tensor_copy(den_sb[:], den[:])
                            next_den = pde.tile([P, 4], F32, tag="de", name="de")
                            nc.vector.memset(next_den[:], 0.0)
                            tpd = pde.tile([4, P], F32R, tag="tpd", bufs=1,
                                           name="tpd")
                            nc.tensor.transpose(tpd[0:4, 0:P], den_sb[:], ident[:])
                            rec = pat.tile([4, P], F32, tag="rec", bufs=2,
                                           name="rec")
                            nc.vector.reciprocal(rec[:], tpd[:])
                            recb = pat.tile([4, P], BF16, tag="recb", bufs=2,
                                            name="recb")
                            nc.vector.tensor_copy(recb[:], rec[:])
                            oh = pat.tile([P, TCH], BF16, tag=f"oh{h}", bufs=2,
                                          name=f"oh{h}")
                            ohs.append(oh)

                            def norm_tail(recb=recb, outU=outU, oh=oh, h=h,
                                          tq=tq):
                                bc = pmm.tile([P, TCH], F32, tag="mm", name="mm")
                                for qq in range(4):
                                    nc.tensor.matmul(
                                        bc[:, qq * P:(qq + 1) * P],
                                        sel4[:, qq * P:(qq + 1) * P],
                                        recb[:],
                                        start=True, stop=True,
                                        skip_group_check=True,
                                    )
                                bcs = pat.tile([P, TCH], BF16, tag="bcs",
                                               bufs=2, name="bcs")
                                nc.vector.tensor_copy(bcs[:], bc[:])
                                nc.vector.tensor_mul(oh[:], outU[:], bcs[:])
                                if h == HLOC - 1:
                                    for cs0 in range(NCT):
                                        outproj_q.append(
                                            (ohs_by_tq[tq], tq, cs0))

                            deferred_b.append(norm_tail)

                    # drain deferred work
                    while deferred_b:
                        deferred_b.popleft()()
                    while outproj_q:
                        emit_outproj_group()

                pb2_ctx.__exit__(None, None, None)

    nc.compile()
    return nc


def _get_nc():
    if "nc" not in _NC_CACHE:
        _NC_CACHE["nc"] = build()
    return _NC_CACHE["nc"]


def kernel(x, freqs_cos, freqs_sin, W_dq, W_uq, W_dkv, W_uk, W_uv, W_qr, W_kr,
           W_o, trace=False, **trace_kwargs):
    nc = _get_nc()
    f32 = lambda a: np.ascontiguousarray(np.asarray(a, dtype=np.float32))
    x = f32(x); W_dq = f32(W_dq); W_uq = f32(W_uq); W_dkv = f32(W_dkv)
    W_uk = f32(W_uk); W_uv = f32(W_uv); W_qr = f32(W_qr); W_kr = f32(W_kr)
    W_o = f32(W_o)
    cos = f32(freqs_cos); sin = f32(freqs_sin)

    in_maps = []
    for c in range(8):
        b, r = divmod(c, 4)
        in_maps.append({
            "x": x[b],
            "wdq": W_dq, "wdkv": W_dkv, "wkr": W_kr,
            "wuq": W_uq[r * HLOC * HS:(r + 1) * HLOC * HS],
            "wuk": W_uk[r * HLOC * HS:(r + 1) * HLOC * HS],
            "wuv": W_uv[r * HLOC * HS:(r + 1) * HLOC * HS],
            "wqr": W_qr[r * HLOC * RHD:(r + 1) * HLOC * RHD],
            "wo": W_o[:, r * HLOC * HS:(r + 1) * HLOC * HS],
            "cos": cos, "sin": sin,
        })
    res = run_bass_kernel_spmd(nc, in_maps, core_ids=list(range(8)),
                               trace=trace, **trace_kwargs)
    out = np.zeros((B, T, C), dtype=np.float32)
    for c in range(8):
        b = c // 4
        out[b] += res.results[c]["out"].T
    kernel.last_result = res
    return out


# revision 48
# speedup vs baseline: 1.1768x; 1.0983x over previous
"""MLA-style attention kernel for 8 TRN2 NeuronCores, v3.

Sharding: core c -> batch b = c//4, heads r*4..r*4+3 where r = c%4.
The latent down-projections are REPLICATED within each 4-core batch
group (no collective, no cross-core dependency): each core computes the
full-T latents c_q/c_kv/k_r from the full x[b], then its 4 heads'
attention and a partial output projection summed on the host.

All activations stay SBUF-resident in a transposed [feature, T] layout.
Down/up-projections run in bf16 (PSUM fp32 accumulate).  Scores use
fp8e4 with DoubleRow perf mode: q/k packed as [128, 2, T] fp8 where
slot 0 holds the 128 content dims and slot 1 rows 0:64 hold the roped
rope dims (planar re/im), rows 64:128 zero.  One DoubleRow matmul per
512x128 score subtile (4x fewer PE cycles than two f32r matmuls).
Softmax denominators use N=1 ones-column matmuls accumulating into a
[128q, 4] PSUM tile (start=False onto memset zeros -- a start=True
would wipe sibling columns through the 2KB zero-region), then
transpose + reciprocal + selector broadcast matmul for normalization.
Diagonal causal blocks shrink the score matmul to the valid q-range,
memset the dead Pt columns, and add a fixed 128-wide triangular mask.
exp() pipelines two subtiles deep; the normalization tail of head h and
the output projection of chunk tq-1 are emitted under the NEXT head's
score loop so PE never drains while Act works.  V is produced directly
in natural [t, hs] layout (lhsT = ckv^T): no transposes after phase A.
Both hardware DMA queues are used: SP for W_dq/W_dkv/x/output, Act for
cos/sin, W_u/W_qr/W_o, mask, and the SBUF-to-SBUF fp8 slot copies.
"""
import math
from collections import deque
import numpy as np

import concourse.bass as bass
import concourse.bacc as bacc
import concourse.mybir as mybir
import concourse.tile as tile
from concourse.bass_utils import run_bass_kernel_spmd

F32 = mybir.dt.float32
F32R = mybir.dt.float32r
BF16 = mybir.dt.bfloat16
F8 = mybir.dt.float8e4
Exp = mybir.ActivationFunctionType.Exp
DR = mybir.MatmulPerfMode.DoubleRow

B, T, C = 2, 2048, 2048
H = 16
HS = 128
NL = 512
RHD = 64
HLOC = 4              # heads per core
P = 128
NNL = NL // P         # 4
NCT = C // P          # 16
TCH = 512
NCH = T // TCH        # 4
SCALE = 1.0 / math.sqrt(HS + RHD)
NEG = -1.0e30

_NC_CACHE = {}


def _deint(ap2d):
    # [p, 2d] -> (evens [p, d], odds [p, d]) along the free dim
    rr = ap2d.rearrange("p (d two) -> p two d", two=2)
    return rr[:, 0, :], rr[:, 1, :]


def build():
    nc = bacc.Bacc("TRN2", target_bir_lowering=False, debug=False, num_devices=8)

    x_ext = nc.dram_tensor("x", [T, C], F32R, kind="ExternalInput")
    wdq_ext = nc.dram_tensor("wdq", [NL, C], F32R, kind="ExternalInput")
    wdkv_ext = nc.dram_tensor("wdkv", [NL, C], F32R, kind="ExternalInput")
    wkr_ext = nc.dram_tensor("wkr", [RHD, C], F32R, kind="ExternalInput")
    wuq_ext = nc.dram_tensor("wuq", [HLOC * HS, NL], F32R, kind="ExternalInput")
    wuk_ext = nc.dram_tensor("wuk", [HLOC * HS, NL], F32R, kind="ExternalInput")
    wuv_ext = nc.dram_tensor("wuv", [HLOC * HS, NL], F32R, kind="ExternalInput")
    wqr_ext = nc.dram_tensor("wqr", [HLOC * RHD, NL], F32R, kind="ExternalInput")
    wo_ext = nc.dram_tensor("wo", [C, HLOC * HS], F32R, kind="ExternalInput")
    cos_ext = nc.dram_tensor("cos", [T, RHD // 2], F32R, kind="ExternalInput")
    sin_ext = nc.dram_tensor("sin", [T, RHD // 2], F32R, kind="ExternalInput")
    out_ext = nc.dram_tensor("out", [C, T], F32, kind="ExternalOutput")
    xown_ext = nc.dram_tensor("xown", [TCH, C], F32R, kind="ExternalInput")
    agin_dram = nc.dram_tensor("agin", [NL + RHD, TCH], BF16)
    agout_dram = nc.dram_tensor("agout", [4, NL + RHD, TCH], BF16)

    ident_dram = nc.inline_tensor(np.eye(P, dtype=np.float32), name="identc")
    # triangular mask for the 128-wide diagonal band of shrunk S^T tiles
    m = np.full((P, 896), NEG, dtype=np.float32)
    for jj in range(P):
        m[jj, 384 + jj:] = 0.0
    masks_dram = nc.inline_tensor(m, name="maskc")
    # row selector for the 1/den broadcast: sel4[k, qq*128+j] = (k == qq)
    sel = np.zeros((4, 512), dtype=np.float32)
    for qq in range(4):
        sel[qq, qq * P:(qq + 1) * P] = 1.0
    sel4_dram = nc.inline_tensor(sel, name="sel4c")

    with tile.TileContext(nc) as tc:
        with tc.tile_pool(name="pers", bufs=1) as pers:
            ptp_cell = [None]
            ident = pers.tile([P, P], F32R, tag="ident", name="ident")
            nc.sync.dma_start(ident[:], ident_dram.ap().bitcast(F32R))
            maskbuf = pers.tile([P, 896], BF16, tag="maskbuf", name="maskbuf")
            nc.gpsimd.dma_start(out=maskbuf[:], in_=masks_dram.ap())
            onescol = pers.tile([P, 1], BF16, tag="onescol", name="onescol")
            nc.vector.memset(onescol[:], 1.0)
            identb = pers.tile([P, P], BF16, tag="identb", name="identb")
            nc.vector.tensor_copy(identb[:], ident[:])
            sel4 = pers.tile([4, TCH], BF16, tag="sel4", name="sel4")
            nc.gpsimd.dma_start(out=sel4[:], in_=sel4_dram.ap())

            # rope tables, cos/sin duplicated on all four 32-row groups
            ca4 = pers.tile([P, T], BF16, tag="ca4", name="ca4")
            sa4 = pers.tile([P, T], BF16, tag="sa4", name="sa4")

            # full-T latents (bf16, [feat, T])
            cqT = [pers.tile([P, T], BF16, tag=f"cqT{i}", name=f"cqT{i}")
                   for i in range(NNL)]
            ckvT = [pers.tile([P, T], BF16, tag=f"ckvT{i}", name=f"ckvT{i}")
                    for i in range(NNL)]
            krraw = pers.tile([RHD, T], BF16, tag="krraw", name="krraw")

            _ecnt = [0]

            def ecopy(dst, src, pin=None):
                """PSUM->SBUF evacuation copy, alternating Act/DVE."""
                _ecnt[0] += 1
                eng = pin if pin else ("act" if _ecnt[0] % 2 else "dve")
                if eng == "act":
                    nc.scalar.copy(dst, src)
                else:
                    nc.vector.tensor_copy(dst, src)

            def transpose_pair_into(dst_ap, srcA, srcB, pin=None):
                bf = srcA.dtype == BF16
                tp2 = ptp_cell[0].tile([P, 2 * P], BF16 if bf else F32R,
                                       tag="tpb" if bf else "tp", name="tp")
                idn = identb if bf else ident
                nc.tensor.transpose(tp2[:, 0:P], srcA, idn[:])
                nc.tensor.transpose(tp2[:, P:2 * P], srcB, idn[:])
                ecopy(dst_ap, tp2[:], pin=pin)

            # ============ phase A: x chunks + all weight prep, interleaved ==
            with tc.tile_pool(name="pb", bufs=1) as pb:
                # -- persistent-ish weight destinations (pb outlives phase A)
                wuqT = [pb.tile([P, HLOC * HS], BF16, tag=f"wuqT{i}",
                                name=f"wuqT{i}") for i in range(NNL)]
                wukT = [pb.tile([P, HLOC * HS], BF16, tag=f"wukT{i}",
                                name=f"wukT{i}") for i in range(NNL)]
                wuvT = [pb.tile([P, HLOC * HS], BF16, tag=f"wuvT{i}",
                                name=f"wuvT{i}") for i in range(NNL)]
                wqrT = [[pb.tile([P, P], BF16, tag=f"wqrT{g}{i}",
                                 name=f"wqrT{g}{i}") for i in range(NNL)]
                        for g in range(2)]
                woT = [pb.tile([P, C], BF16, tag=f"woT{i}", name=f"woT{i}")
                       for i in range(HLOC)]

                pa_ctx = (
                    tc.tile_pool(name="pa", bufs=1),
                    tc.tile_pool(name="pacc", bufs=1, space="PSUM"),
                    tc.tile_pool(name="ptpA", bufs=3, space="PSUM"),
                )
                pa = pa_ctx[0].__enter__()
                pacc = pa_ctx[1].__enter__()
                ptp_cell[0] = pa_ctx[2].__enter__()

                wdqT = [pa.tile([P, NL], BF16, tag=f"wdqT{i}", name=f"wdqT{i}")
                        for i in range(NCT)]
                wdkvT = [pa.tile([P, NL], BF16, tag=f"wdkvT{i}",
                                 name=f"wdkvT{i}") for i in range(NCT)]
                wkrT = [pa.tile([P, RHD], BF16, tag=f"wkrT{i}", name=f"wkrT{i}")
                        for i in range(NCT)]
                xT = [pa.tile([P, TCH], BF16, tag=f"xT{i}", name=f"xT{i}")
                      for i in range(NCT)]

                def x_chunk_transpose(tch):
                    t0 = tch * TCH
                    for hf in range(2):
                        cf = slice(hf * (C // 2), (hf + 1) * (C // 2))
                        xab = pa.tile([P, 4, C // 2], BF16, tag="xab", bufs=3,
                                      name="xab")
                        nc.gpsimd.dma_start(
                            out=xab[:],
                            in_=x_ext.ap()[t0:t0 + TCH, cf].rearrange(
                                "(s p) c -> p s c", p=P))
                        for sp in range(2):
                            for ci in range(NCT // 2):
                                transpose_pair_into(
                                    xT[hf * 8 + ci][:, 2 * sp * P:(2 * sp + 2) * P],
                                    xab[:, 2 * sp, ci * P:(ci + 1) * P],
                                    xab[:, 2 * sp + 1, ci * P:(ci + 1) * P],
                                )

                def x_chunk_matmuls(tch):
                    t0 = tch * TCH
                    for j in range(NNL):
                        acc = pacc.tile([P, TCH], F32, tag=f"acc{j}",
                                        name=f"acc{j}")
                        for ci in range(NCT):
                            nc.tensor.matmul(
                                acc[:],
                                wdqT[ci][:, j * P:(j + 1) * P],
                                xT[ci][:],
                                start=(ci == 0),
                                stop=(ci == NCT - 1),
                            )
                        ecopy(cqT[j][:, t0:t0 + TCH], acc[:])

                def own_prep():
                    # own T-chunk: ckv + kr from xown, shared via AllGather
                    for hf in range(2):
                        cf = slice(hf * (C // 2), (hf + 1) * (C // 2))
                        xab = pa.tile([P, 4, C // 2], BF16, tag="xab", bufs=3,
                                      name="xab")
                        nc.gpsimd.dma_start(
                            out=xab[:],
                            in_=xown_ext.ap()[:, cf].rearrange(
                                "(s p) c -> p s c", p=P))
                        for sp in range(2):
                            for ci in range(NCT // 2):
                                transpose_pair_into(
                                    xT[hf * 8 + ci][:, 2 * sp * P:(2 * sp + 2) * P],
                                    xab[:, 2 * sp, ci * P:(ci + 1) * P],
                                    xab[:, 2 * sp + 1, ci * P:(ci + 1) * P],
                                )
                    for j in range(NNL):
                        acc = pacc.tile([P, TCH], F32, tag=f"acc{j}",
                                        name=f"acc{j}")
                        for ci in range(NCT):
                            nc.tensor.matmul(
                                acc[:],
                                wdkvT[ci][:, j * P:(j + 1) * P],
                                xT[ci][:],
                                start=(ci == 0),
                                stop=(ci == NCT - 1),
                            )
                        agst = pa.tile([P, TCH], BF16, tag="agst", bufs=2,
                                       name="agst")
                        ecopy(agst[:], acc[:], pin="act")
                        nc.sync.dma_start(
                            agin_dram.ap()[j * P:(j + 1) * P, :], agst[:])
                    acck = pacc.tile([RHD, TCH], F32, tag="acck", name="acck")
                    for ci in range(NCT):
                        nc.tensor.matmul(
                            acck[:],
                            wkrT[ci][:],
                            xT[ci][:],
                            start=(ci == 0),
                            stop=(ci == NCT - 1),
                        )
                    agsk = pa.tile([RHD, TCH], BF16, tag="agsk", name="agsk")
                    ecopy(agsk[:], acck[:], pin="act")
                    nc.sync.dma_start(agin_dram.ap()[NL:NL + RHD, :], agsk[:])
                    nc.gpsimd.collective_compute(
                        "AllGather",
                        mybir.AluOpType.bypass,
                        replica_groups=[[0, 1, 2, 3], [4, 5, 6, 7]],
                        ins=[agin_dram.ap().opt()],
                        outs=[agout_dram.ap().opt()],
                    )

                def unpack_gather():
                    for p_ in range(4):
                        sl = slice(p_ * TCH, (p_ + 1) * TCH)
                        for j in range(NNL):
                            nc.sync.dma_start(
                                ckvT[j][:, sl],
                                agout_dram.ap()[p_, j * P:(j + 1) * P, :])
                        nc.sync.dma_start(
                            krraw[:, sl],
                            agout_dram.ap()[p_, NL:NL + RHD, :])

                def wd_one(w_ext, wTs):
                    if True:
                        for hf in range(2):
                            cf = slice(hf * (C // 2), (hf + 1) * (C // 2))
                            sab = pa.tile([P, 4, C // 2], BF16, tag="wsab",
                                          bufs=2, name="wsab")
                            nc.gpsimd.dma_start(
                                out=sab[:],
                                in_=w_ext.ap()[:, cf].rearrange(
                                    "(s p) c -> p s c", p=P))
                            for rp in range(NL // P // 2):
                                for ci in range(NCT // 2):
                                    transpose_pair_into(
                                        wTs[hf * 8 + ci][:, 2 * rp * P:(2 * rp + 2) * P],
                                        sab[:, 2 * rp, ci * P:(ci + 1) * P],
                                        sab[:, 2 * rp + 1, ci * P:(ci + 1) * P],
                                    )
                def wkr_prep():
                    kstrip = pa.tile([RHD, C], BF16, tag="kstrip", name="kstrip")
                    nc.gpsimd.dma_start(out=kstrip[:], in_=wkr_ext.ap())
                    for ci in range(NCT):
                        tp = ptp_cell[0].tile([P, 2 * P], BF16, tag="tpb",
                                              name="tp")
                        nc.tensor.transpose(
                            tp[:, :RHD], kstrip[:, ci * P:(ci + 1) * P],
                            identb[:RHD, :RHD])
                        ev, od = _deint(tp[:, :RHD])
                        nc.scalar.copy(wkrT[ci][:, 0:32], ev)
                        nc.scalar.copy(wkrT[ci][:, 32:64], od)

                def table_prep():
                    cst = pa.tile([P, T // P, RHD // 2], BF16, tag="cst",
                                  name="cst")
                    sst = pa.tile([P, T // P, RHD // 2], BF16, tag="sst",
                                  name="sst")
                    nc.gpsimd.dma_start(
                        out=cst[:],
                        in_=cos_ext.ap().rearrange("(s p) f -> p s f", p=P))
                    nc.gpsimd.dma_start(
                        out=sst[:],
                        in_=sin_ext.ap().rearrange("(s p) f -> p s f", p=P))
                    for s in range(T // P):
                        tp = ptp_cell[0].tile([P, 2 * P], BF16, tag="tpb",
                                              name="tp")
                        nc.tensor.transpose(tp[:32, 0:P], cst[:, s, :], identb[:])
                        nc.tensor.transpose(tp[:32, P:2 * P], sst[:, s, :],
                                            identb[:])
                        nc.vector.tensor_copy(ca4[0:32, s * P:(s + 1) * P],
                                              tp[:32, 0:P])
                        nc.vector.tensor_copy(sa4[0:32, s * P:(s + 1) * P],
                                              tp[:32, P:2 * P])
                    for d in range(1, 4):
                        nc.vector.tensor_copy(ca4[32 * d:32 * (d + 1), :],
                                              ca4[0:32, :])
                        nc.vector.tensor_copy(sa4[32 * d:32 * (d + 1), :],
                                              sa4[0:32, :])

                def wu_prep():
                    for w_ext, wT in ((wuq_ext, wuqT), (wuk_ext, wukT),
                                      (wuv_ext, wuvT)):
                        uab = pa.tile([P, 4, NL], BF16, tag="usab", bufs=2,
                                      name="usab")
                        nc.gpsimd.dma_start(
                            out=uab[:],
                            in_=w_ext.ap().rearrange("(s p) c -> p s c", p=P))
                        for rp in range(HLOC * HS // P // 2):
                            for cs in range(NNL):
                                transpose_pair_into(
                                    wT[cs][:, 2 * rp * P:(2 * rp + 2) * P],
                                    uab[:, 2 * rp, cs * P:(cs + 1) * P],
                                    uab[:, 2 * rp + 1, cs * P:(cs + 1) * P],
                                )

                def wo_wqr_prep():
                    qab = pa.tile([P, 2, NL], BF16, tag="qrs", name="qrs")
                    nc.gpsimd.dma_start(
                        out=qab[:],
                        in_=wqr_ext.ap().rearrange("(s p) c -> p s c", p=P))
                    for g in range(2):
                        strip = qab[:, g, :]
                        for cs in range(NNL):
                            tp = ptp_cell[0].tile([P, 2 * P], BF16, tag="tpb",
                                                  name="tp")
                            nc.tensor.transpose(
                                tp[:, 0:P], strip[:, cs * P:(cs + 1) * P],
                                identb[:])
                            evA, odA = _deint(tp[:, 0:RHD])
                            evB, odB = _deint(tp[:, RHD:2 * RHD])
                            nc.scalar.copy(wqrT[g][cs][:, 0:32], evA)
                            nc.scalar.copy(wqrT[g][cs][:, 32:64], evB)
                            nc.scalar.copy(wqrT[g][cs][:, 64:96], odA)
                            nc.scalar.copy(wqrT[g][cs][:, 96:128], odB)
                    for half in range(2):
                        oab = pa.tile([P, 8, HLOC * HS], BF16, tag="osab",
                                      bufs=2, name="osab")
                        nc.gpsimd.dma_start(
                            out=oab[:],
                            in_=wo_ext.ap()[half * (C // 2):(half + 1) * (C // 2), :]
                            .rearrange("(s p) c -> p s c", p=P))
                        for sp2 in range(4):
                            sp = half * 4 + sp2
                            for fs in range(HLOC):
                                transpose_pair_into(
                                    woT[fs][:, 2 * sp * P:(2 * sp + 2) * P],
                                    oab[:, 2 * sp2, fs * P:(fs + 1) * P],
                                    oab[:, 2 * sp2 + 1, fs * P:(fs + 1) * P],
                                )

                # own-chunk ckv/kr first so the AllGather hides under the
                # local c_q down-projection and weight prep.
                wd_one(wdkv_ext, wdkvT)
                wkr_prep()
                own_prep()
                wd_one(wdq_ext, wdqT)
                x_chunk_transpose(0)
                x_chunk_matmuls(0)
                x_chunk_transpose(1)
                table_prep()
                x_chunk_matmuls(1)
                x_chunk_transpose(2)
                wu_prep()
                x_chunk_matmuls(2)
                x_chunk_transpose(3)
                wo_wqr_prep()
                x_chunk_matmuls(3)
                unpack_gather()

                pa_ctx[2].__exit__(None, None, None)
                pa_ctx[1].__exit__(None, None, None)
                pa_ctx[0].__exit__(None, None, None)

                # ============ phase B: rope, up-projections, fp8 packs ======
                pb2_ctx = tc.tile_pool(name="pb2", bufs=1)
                pb2 = pb2_ctx.__enter__()
                q8 = [pb2.tile([P, 2, T], F8, tag=f"q8{h}", name=f"q8{h}")
                      for h in range(HLOC)]
                k8 = [pb2.tile([P, 2, T], F8, tag=f"k8{h}", name=f"k8{h}")
                      for h in range(HLOC)]
                vv = [pb2.tile([P, T // P, P], BF16, tag=f"vv{h}", name=f"vv{h}")
                      for h in range(HLOC)]
                for h in range(HLOC):
                    nc.vector.memset(q8[h][64:128, 1, :], 0.0)
                    nc.gpsimd.memset(k8[h][64:128, 1, :], 0.0)

                with (
                    tc.tile_pool(name="pmm", bufs=3, space="PSUM") as pmm,
                    tc.tile_pool(name="pou", bufs=2, space="PSUM") as pou,
                    tc.tile_pool(name="pde", bufs=2, space="PSUM") as pde,
                    tc.tile_pool(name="pat", bufs=1) as pat,
                ):
                    # k_r rope -> krf8 (planar re/im), shared across heads
                    krf8 = pb2.tile([RHD, T], F8, tag="krf8", name="krf8")
                    rtmp = pb2.tile([P, T], BF16, tag="rtmp", name="rtmp")
                    rro = pb2.tile([P, T], BF16, tag="rro", name="rro")
                    nc.vector.tensor_mul(rtmp[0:32, :], krraw[32:64, :], sa4[32:64, :])
                    nc.vector.tensor_mul(rtmp[32:64, :], krraw[32:64, :], ca4[32:64, :])
                    nc.vector.tensor_mul(rro[0:32, :], krraw[0:32, :], ca4[0:32, :])
                    nc.vector.tensor_mul(rro[32:64, :], krraw[0:32, :], sa4[0:32, :])
                    nc.vector.tensor_sub(krf8[0:32, :], rro[0:32, :], rtmp[0:32, :])
                    nc.vector.tensor_add(krf8[32:64, :], rro[32:64, :], rtmp[32:64, :])
                    for h in range(HLOC):
                        nc.sync.dma_start(k8[h][0:RHD, 1, :], krf8[:])

                    # up-projections, head-pair at a time
                    for g in range(2):
                        hA, hB = 2 * g, 2 * g + 1
                        qraw = pb2.tile([P, T], BF16, tag="qraw", name="qraw")
                        for ch in range(NCH):
                            sl = slice(ch * TCH, (ch + 1) * TCH)
                            for hh in (hA, hB):
                                for wT, src, dst in ((wuqT, cqT, q8),
                                                     (wukT, ckvT, k8)):
                                    acc = pmm.tile([P, TCH], F32, tag="mm",
                                                   name="mm")
                                    for nl in range(NNL):
                                        nc.tensor.matmul(
                                            acc[:],
                                            wT[nl][:, hh * P:(hh + 1) * P],
                                            src[nl][:, sl],
                                            start=(nl == 0),
                                            stop=(nl == NNL - 1),
                                        )
                                    ecopy(dst[hh][:, 0, sl], acc[:], pin="act")
                            qacc = pmm.tile([P, TCH], F32, tag="mm", name="mm")
                            for nl in range(NNL):
                                nc.tensor.matmul(
                                    qacc[:],
                                    wqrT[g][nl][:],
                                    cqT[nl][:, sl],
                                    start=(nl == 0),
                                    stop=(nl == NNL - 1),
                                )
                            ecopy(qraw[:, sl], qacc[:], pin="act")
                        # rope for the pair: rows [Are, Bre, Aim, Bim]
                        roq = pb2.tile([P, T], F8, tag="roq", bufs=2, name="roq")
                        nc.vector.tensor_mul(rtmp[0:64, :], qraw[64:128, :],
                                             sa4[64:128, :])
                        nc.vector.tensor_mul(rtmp[64:128, :], qraw[64:128, :],
                                             ca4[64:128, :])
                        nc.vector.tensor_mul(rro[0:64, :], qraw[0:64, :],
                                             ca4[0:64, :])
                        nc.vector.tensor_mul(rro[64:128, :], qraw[0:64, :],
                                             sa4[0:64, :])
                        nc.vector.tensor_sub(roq[0:64, :], rro[0:64, :],
                                             rtmp[0:64, :])
                        nc.vector.tensor_add(roq[64:128, :], rro[64:128, :],
                                             rtmp[64:128, :])
                        nc.sync.dma_start(q8[hA][0:32, 1, :], roq[0:32, :])
                        nc.sync.dma_start(q8[hA][32:64, 1, :], roq[64:96, :])
                        nc.sync.dma_start(q8[hB][0:32, 1, :], roq[32:64, :])
                        nc.sync.dma_start(q8[hB][32:64, 1, :], roq[96:128, :])
                        # natural-layout V for both heads
                        for hh in (hA, hB):
                            for ts4 in range(T // TCH):
                                vps = pmm.tile([P, TCH], F32, tag="mm", name="mm")
                                for j in range(4):
                                    kt = ts4 * 4 + j
                                    for nl in range(NNL):
                                        nc.tensor.matmul(
                                            vps[:, j * P:(j + 1) * P],
                                            ckvT[nl][:, kt * P:(kt + 1) * P],
                                            wuvT[nl][:, hh * P:(hh + 1) * P],
                                            start=(nl == 0),
                                            stop=(nl == NNL - 1),
                                            skip_group_check=True,
                                        )
                                ecopy(
                                    vv[hh][:, ts4 * 4:(ts4 + 1) * 4, :],
                                    vps[:].rearrange("p (a b) -> p a b", a=4),
                                    pin="act",
                                )

                    # ============ attention + deferred norm/output proj =====
                    deferred_b = deque()   # normalization tails
                    outproj_q = deque()    # (ohs, tq, cs_start) groups

                    def emit_outproj_group():
                        g_ohs, g_tq, cs = outproj_q.popleft()
                        g_qsl = slice(g_tq * TCH, (g_tq + 1) * TCH)
                        acc = pmm.tile([P, TCH], F32, tag="mm", name="mm")
                        for h2 in range(HLOC):
                            nc.tensor.matmul(
                                acc[:],
                                woT[h2][:, cs * P:(cs + 1) * P],
                                g_ohs[h2][:],
                                start=(h2 == 0),
                                stop=(h2 == HLOC - 1),
                            )
                        ot = pat.tile([P, TCH], F32, tag="ot", bufs=3,
                                      name="ot")
                        ecopy(ot[:], acc[:], pin="dve")
                        nc.sync.dma_start(
                            out_ext.ap()[cs * P:(cs + 1) * P, g_qsl], ot[:])

                    ohs_by_tq = {}
                    next_den = pde.tile([P, 4], F32, tag="de", name="de")
                    nc.vector.memset(next_den[:], 0.0)
                    for tq in range(NCH):
                        qsl = slice(tq * TCH, (tq + 1) * TCH)
                        ohs = []
                        ohs_by_tq[tq] = ohs
                        for h in range(HLOC):
                            outU = pou.tile([P, TCH], F32, tag="ou", name="ou")
                            den = next_den
                            nsub = (tq + 1) * 4
                            pend = deque()

                            def flush_one(outU=outU, den=den, nsub=nsub, h=h,
                                          tq=tq, pend=pend):
                                Pt, kt = pend.popleft()
                                kc, ks = kt // 4, kt % 4
                                off = ks * P if kc == tq else 0
                                for qq in range(off // P, 4):
                                    nc.tensor.matmul(
                                        den[:, qq:qq + 1],
                                        Pt[:, qq * P:(qq + 1) * P],
                                        onescol[:],
                                        start=False,
                                        stop=(kt == nsub - 1),
                                        skip_group_check=True,
                                    )
                                nc.tensor.matmul(
                                    outU[:, off:TCH],
                                    vv[h][:, kt, :],
                                    Pt[:, off:TCH],
                                    start=(kt == 0),
                                    stop=(kt == nsub - 1),
                                    skip_group_check=True,
                                )

                            for kt in range(nsub):
                                kc, ks = kt // 4, kt % 4
                                diag = kc == tq
                                off = ks * P if diag else 0
                                npr = TCH - off
                                ST = pmm.tile([P, TCH], F32, tag="mm", name="mm")
                                nc.tensor.matmul(
                                    ST[:, 0:npr],
                                    k8[h][:, :, kt * P:(kt + 1) * P],
                                    q8[h][:, :, qsl.start + off:qsl.stop],
                                    perf_mode=DR,
                                    start=True,
                                    stop=not diag,
                                )
                                if diag:
                                    # causal band mask folded in on PE:
                                    # ST[:, 0:128] += I^T @ mask
                                    nc.tensor.matmul(
                                        ST[:, 0:P],
                                        identb[:],
                                        maskbuf[:, 384:384 + P],
                                        start=False,
                                        stop=True,
                                        skip_group_check=True,
                                    )
                                Pt = pat.tile([P, TCH], BF16, tag="pt", bufs=8,
                                              name="pt")
                                nc.scalar.activation(Pt[:, off:TCH], ST[:, 0:npr],
                                                     Exp, scale=SCALE)
                                pend.append((Pt, kt))
                                if len(pend) > 3:
                                    flush_one()
                                if kt == 2 and deferred_b:
                                    # previous head's deferred normalization
                                    deferred_b.popleft()()
                                if kt >= 2 and kt % 2 == 0 and outproj_q:
                                    # one 128-row output-projection column
                                    emit_outproj_group()
                            while pend:
                                flush_one()

                            # normalization head: transpose+recip now (DVE),
                            # broadcast+apply deferred under the next head
                            den_sb = pat.tile([P, 4], F32R, tag="dsb", bufs=2,
                                              name="dsb")
                            nc.vector.tensor_copy(den_sb[:], den[:])
                            next_den = pde.tile([P, 4], F32, tag="de", name="de")
                            nc.vector.memset(next_den[:], 0.0)
                            tpd = pde.tile([4, P], F32R, tag="tpd", bufs=1,
                                           name="tpd")
                            nc.tensor.transpose(tpd[0:4, 0:P], den_sb[:], ident[:])
                            rec = pat.tile([4, P], F32, tag="rec", bufs=2,
                                           name="rec")
                            nc.vector.reciprocal(rec[:], tpd[:])
                            recb = pat.tile([4, P], BF16, tag="recb", bufs=2,
                                            name="recb")
                            nc.vector.tensor_copy(recb[:], rec[:])
                            oh = pat.tile([P, TCH], BF16, tag=f"oh{h}", bufs=2,
                                          name=f"oh{h}")
                            ohs.append(oh)

                            def norm_tail(recb=recb, outU=outU, oh=oh, h=h,
                                          tq=tq):
                                bc = pmm.tile([P, TCH], F32, tag="mm", name="mm")
                                for qq in range(4):
                                    nc.tensor.matmul(
                                        bc[:, qq * P:(qq + 1) * P],
                                        sel4[:, qq * P:(qq + 1) * P],
                                        recb[:],
                                        start=True, stop=True,
                                        skip_group_check=True,
                                    )
                                bcs = pat.tile([P, TCH], BF16, tag="bcs",
                                               bufs=2, name="bcs")
                                nc.vector.tensor_copy(bcs[:], bc[:])
                                nc.vector.tensor_mul(oh[:], outU[:], bcs[:])
                                if h == HLOC - 1:
                                    for cs0 in range(NCT):
                                        outproj_q.append(
                                            (ohs_by_tq[tq], tq, cs0))

                            deferred_b.append(norm_tail)

                    # drain deferred work
                    while deferred_b:
                        deferred_b.popleft()()
                    while outproj_q:
                        emit_outproj_group()

                pb2_ctx.__exit__(None, None, None)

    nc.compile()
    return nc


def _get_nc():
    if "nc" not in _NC_CACHE:
        _NC_CACHE["nc"] = build()
    return _NC_CACHE["nc"]


def kernel(x, freqs_cos, freqs_sin, W_dq, W_uq, W_dkv, W_uk, W_uv, W_qr, W_kr,
           W_o, trace=False, **trace_kwargs):
    nc = _get_nc()
    f32 = lambda a: np.ascontiguousarray(np.asarray(a, dtype=np.float32))
    x = f32(x); W_dq = f32(W_dq); W_uq = f32(W_uq); W_dkv = f32(W_dkv)
    W_uk = f32(W_uk); W_uv = f32(W_uv); W_qr = f32(W_qr); W_kr = f32(W_kr)
    W_o = f32(W_o)
    cos = f32(freqs_cos); sin = f32(freqs_sin)

    in_maps = []
    for c in range(8):
        b, r = divmod(c, 4)
        in_maps.append({
            "x": x[b], "xown": x[b, r * TCH:(r + 1) * TCH],
            "wdq": W_dq, "wdkv": W_dkv, "wkr": W_kr,
            "wuq": W_uq[r * HLOC * HS:(r + 1) * HLOC * HS],
            "wuk": W_uk[r * HLOC * HS:(r + 1) * HLOC * HS],
            "wuv": W_uv[r * HLOC * HS:(r + 1) * HLOC * HS],
            "wqr": W_qr[r * HLOC * RHD:(r + 1) * HLOC * RHD],
            "wo": W_o[:, r * HLOC * HS:(r + 1) * HLOC * HS],
            "cos": cos, "sin": sin,
        })
    res = run_bass_kernel_spmd(nc, in_maps, core_ids=list(range(8)),
                               trace=trace, **trace_kwargs)
    out = np.zeros((B, T, C), dtype=np.float32)
    for c in range(8):
        b = c // 4
        out[b] += res.results[c]["out"].T
    kernel.last_result = res
    return out
